# revision 36
# baseline (speedup 1.0000x reference)
"""Block-causal GQA attention on 8 trn2 NeuronCores.

Sharding: core = b*4 + g  (b in {0,1} batch, g in {0..3} kv-head group).
Each core computes, for its batch b and kv group g (4 q-heads, 1 kv head):
    partial_out = softmax_blockcausal(rope(x@Wq_g) @ rope(x@Wk_g)^T) @ (x@Wv_g) @ Wo_g
Host sums the 4 group partials per batch.

Device design (bf16 matmuls, f32 PSUM):
  - Host passes x^T, so Q^T/K^T/V^T come out of projections with d on
    partitions and no on-device transposes; RoPE (sign folded into the sin
    table) happens on DVE during PSUM eviction.  V^T is DMA-xbar-transposed
    into V_aug = [V | ones].
  - Projections run c-chunk-outer in PSUM waves (K+V, Q0+Q1, Q2+Q3) so PE
    work starts as soon as the first x^T chunk lands.
  - Attention per (head, 1024-wide tq half): S^T[tk,tq] = K^T.T @ Q^T,
    exp on ACT (scale=1/sqrt(128); scores are O(1) so no max subtraction),
    then per 128-wide tq tile: [Y|Z][tq,129] += P^T_tile.T @ V_aug
    (P^T stationary, fused softmax denominator in column 128).
    Normalize: rz = 1/Z [tq,1], Y *= rz via per-partition tensor_scalar,
    DMA-transpose Y tile into Y^T[d, tq].  Half-0 score matmuls compute
    the masked (junk) columns too so PE/HAM stay busy under the
    ACT-bound exp stretch.
  - Tail: the last head's half-1 Y accumulates V-stationary straight
    into Y^T psum (Z via an all-ones stationary), so no DMA-xbar
    transpose sits between the last exp and the final O-proj rows.
  - O[t,n] = sum_h Y_h^T.T @ Wo_h accumulated in PSUM over heads;
    bf16 partials out (host sums in f32); evictions alternate DVE/ACT,
    tail output drains on two DMA queues.
"""
import os
import sys
from contextlib import ExitStack

import numpy as np

for _p in ("/opt/trn_rl_repo",):
    if _p not in sys.path and os.path.isdir(_p):
        sys.path.insert(0, _p)

import ml_dtypes

BF16 = ml_dtypes.bfloat16

B = 2
T = 2048
C = 2048
HD = 128
NHL = 4           # q heads per core
NT = T // 128     # 16 query/key tiles
NCH = C // 128    # 16 contraction chunks
HW = T // 2       # tq half width
SCALE = 1.0 / float(np.sqrt(np.float32(HD)))

_CACHE = {}


def _build_nc():
    import concourse.bass as bass
    import concourse.mybir as mybir
    import concourse.tile as tile
    from concourse import bacc

    dt = mybir.dt
    f32 = dt.float32
    bf = dt.bfloat16
    Exp = mybir.ActivationFunctionType.Exp

    nc = bacc.Bacc(None, target_bir_lowering=False)

    # weights host-prelaid as [partition, chunk, m] so each DMA is 128 fat
    # contiguous descriptors instead of 2048 small ones
    xT = nc.declare_dram_parameter("xT", [C, T], bf, isOutput=False)
    wq = nc.declare_dram_parameter("wq", [128, NCH, NHL * HD], bf, isOutput=False)
    wk = nc.declare_dram_parameter("wk", [128, NCH, HD], bf, isOutput=False)
    wv = nc.declare_dram_parameter("wv", [128, NCH, HD], bf, isOutput=False)
    wo = nc.declare_dram_parameter("wo", [128, NHL, C], bf, isOutput=False)
    cosT = nc.declare_dram_parameter("cosT", [HD, T], bf, isOutput=False)
    sinT = nc.declare_dram_parameter("sinT", [HD, T], bf, isOutput=False)
    # bf16 partials (host sums in f32): halves output DMA + drain backlog
    o = nc.declare_dram_parameter("o_part", [T, C], bf, isOutput=True)

    with tile.TileContext(nc) as tc:
        with tc.tile_pool(name="consts", bufs=1) as consts:
            # ---- static SBUF loads (order = DMA priority) ----
            wk_sb = consts.tile([128, NCH, HD], bf, name="wk_sb")
            nc.sync.dma_start(wk_sb, wk[:, :, :])
            wv_sb = consts.tile([128, NCH, HD], bf, name="wv_sb")
            nc.sync.dma_start(wv_sb, wv[:, :, :])

            cos_sb = consts.tile([128, T], bf, name="cos_sb")
            sin_sb = consts.tile([128, T], bf, name="sin_sb")
            wq_sb = consts.tile([128, NCH, NHL * HD], bf, name="wq_sb")
            wo_sb = consts.tile([128, NHL, C], bf, name="wo_sb")

            # V_aug = [V | ones]: col 128 preset to 1, cols 0:128 filled by
            # DMA-transpose from V^T after the V projection.  Rows are 256
            # wide so each tile's dst offset stays 512B-aligned — the DMA
            # xbar transpose corrupts data at unaligned dst offsets.
            vaug_sb = consts.tile([128, NT, 2 * HD], bf, name="vaug_sb")
            nc.vector.memset(vaug_sb[:, :, HD:HD + 1], 1.0)

            # all-ones stationary for the tail softmax denominator
            ones_sb = consts.tile([128, 128], bf, name="ones_sb")
            nc.vector.memset(ones_sb, 1.0)

            # warm the ACT exp table set during phase 1
            dumm = consts.tile([1, 8], f32, name="dumm")
            nc.vector.memset(dumm, 0.0)
            nc.scalar.activation(dumm, dumm, Exp)

            # persistent activations
            kt_sb = consts.tile([128, T], bf, name="kt_sb")
            vt_sb = consts.tile([128, T], bf, name="vt_sb")
            qt_sb = [consts.tile([128, T], bf, name=f"qt{h}") for h in range(NHL)]
            yt_sb = [consts.tile([128, T], bf, name=f"yt{h}") for h in range(NHL)]

            # ============ phase 1: K/V projection (c-outer wave) ==========
            # x chunks + rope scratch live on the RIGHT side so the half-1
            # attention pool (opened later, left) can reuse their space
            es1 = ExitStack()
            xtp = es1.enter_context(
                tc.tile_pool(name="xtp", bufs=1, side="right"))
            proj = es1.enter_context(
                tc.tile_pool(name="proj", bufs=1, side="right"))

            xt_r = xT.rearrange("(n p) t -> n p t", p=128)
            xt_sb = []
            for cch in range(NCH):
                xt_c = xtp.tile([128, T], bf, name=f"xt{cch}")
                nc.sync.dma_start(xt_c, xt_r[cch])
                xt_sb.append(xt_c)
                if cch == 10:
                    # wq arrives just before the Q waves need it
                    nc.sync.dma_start(wq_sb, wq[:, :, :])
                elif cch == 12:
                    # rope tables land before the K eviction needs them
                    nc.sync.dma_start(cos_sb, cosT[:, :])
                    nc.sync.dma_start(sin_sb, sinT[:, :])
            nc.sync.dma_start(wo_sb, wo[:, :, :])

            def rope_evict(ps, jsl, dst):
                # dst[:, jsl] = ps * cos + rot_half(ps) * sin  (bf16).
                # ACT does the PSUM eviction; DVE runs at bf16 2x.
                t0 = proj.tile([128, 512], bf, tag="t0", bufs=3)
                t1 = proj.tile([128, 512], bf, tag="t1", bufs=3)
                t2 = proj.tile([128, 512], bf, tag="t2", bufs=3)
                # sin table halves are pre-swapped on host so each mul
                # reads both SBUF inputs at the same base partition
                # (walrus requires equal SBUF base partitions).
                nc.scalar.copy(t0, ps)
                nc.vector.tensor_mul(t1, t0, cos_sb[:, jsl])
                nc.vector.tensor_mul(t2[0:64], t0[64:128], sin_sb[64:128, jsl])
                nc.vector.tensor_mul(t2[64:128], t0[0:64], sin_sb[0:64, jsl])
                nc.vector.tensor_add(dst[:, jsl], t1, t2)

            with tc.tile_pool(name="proj_psum", bufs=2, space="PSUM") as pp:
                # warm the PE clock (HAM) with throwaway matmuls while the
                # input DMAs stream in; results are never read
                warm_ps = pp.tile([128, 512], f32, tag="pj", bufs=8,
                                  name="warm_ps")
                for _ in range(28):
                    nc.tensor.matmul(warm_ps[0:1, :],
                                     vaug_sb[:, 0, HD:HD + 1],
                                     kt_sb[:, 0:512], start=True, stop=True)

                # -- wave 1: K and V (c-outer so PE starts with first chunk) --
                ps_k = [pp.tile([128, 512], f32, tag="pj", bufs=8,
                                name=f"ps_k{j}") for j in range(4)]
                ps_v = [pp.tile([128, 512], f32, tag="pj", bufs=8,
                                name=f"ps_v{j}") for j in range(4)]
                for cch in range(NCH):
                    st, sp = (cch == 0), (cch == NCH - 1)
                    for j in range(T // 512):
                        jsl = slice(512 * j, 512 * (j + 1))
                        nc.tensor.matmul(ps_k[j], wk_sb[:, cch, :],
                                         xt_sb[cch][:, jsl], start=st, stop=sp)
                        nc.tensor.matmul(ps_v[j], wv_sb[:, cch, :],
                                         xt_sb[cch][:, jsl], start=st, stop=sp)
                for j in range(T // 512):
                    jsl = slice(512 * j, 512 * (j + 1))
                    rope_evict(ps_k[j], jsl, kt_sb)
                    nc.scalar.copy(vt_sb[:, jsl], ps_v[j])
                for i in range(NT):
                    nc.sync.dma_start_transpose(
                        vaug_sb[:, i, 0:HD], vt_sb[:, 128 * i:128 * (i + 1)]
                    )

            # ===== phases 1b+2+3: Q proj woven into half-0 attention, =====
            # ===== O-proj rows woven into half-1 attention ================
            # PSUM: tag "s" (2x2 banks) + tag "b1" (3x1, shared by Q-proj
            # accumulators, [Y|Z] groups and O-proj tiles) = 7 banks.
            with tc.tile_pool(name="attn", bufs=1) as ap, \
                 tc.tile_pool(name="attn_psum", bufs=1, space="PSUM") as apsum:

                def q_unit(h, j):
                    hsl = slice(HD * h, HD * (h + 1))
                    jsl = slice(512 * j, 512 * (j + 1))
                    ps_q = apsum.tile([128, 512], f32, tag="b1", bufs=3,
                                      name=f"ps_q{h}_{j}")
                    for cch in range(NCH):
                        nc.tensor.matmul(
                            ps_q, wq_sb[:, cch, hsl], xt_sb[cch][:, jsl],
                            start=(cch == 0), stop=(cch == NCH - 1))
                    rope_evict(ps_q, jsl, qt_sb[h])

                def y_group(h, half, il, tiles):
                    """One [Y|Z] accumulation + normalize + transpose-out."""
                    gi = (HW // 128) * half + il
                    ps_yz = apsum.tile([128, 512], f32, tag="b1", bufs=3,
                                       name="ps_yz")
                    for tk in range(gi + 1):
                        nc.tensor.matmul(
                            ps_yz[:, 0:HD + 1],
                            tiles[tk][:, 128 * il:128 * (il + 1)],
                            vaug_sb[:, tk, 0:HD + 1],
                            start=(tk == 0), stop=(tk == gi))
                    rz = ap.tile([128, 1], f32, tag="rz", bufs=8)
                    nc.vector.reciprocal(rz, ps_yz[:, HD:HD + 1])
                    ysb = ap.tile([128, HD], bf, tag="ysb", bufs=8)
                    nc.vector.tensor_scalar_mul(ysb, ps_yz[:, 0:HD], rz)
                    nc.sync.dma_start_transpose(
                        yt_sb[h][:, 128 * gi:128 * (gi + 1)], ysb)

                def attn_block(h, half, prev, ppool, pbufs, fillers,
                               fill_start, fill_step):
                    """S^T + exp for this tq half.  The previous block's
                    Y-groups plus the filler closures (Q-proj units in half
                    0, O-proj tiles in half 1) are woven between the tk
                    iterations so PE always has queued work while ACT
                    catches up on exps."""
                    tq0 = HW * half
                    ntk = (tq0 + HW) // 128
                    tiles = []
                    pi = 0
                    fi = 0

                    def emit_y(n):
                        nonlocal pi
                        while n > 0 and prev is not None and pi < HW // 128:
                            y_group(prev[0], prev[1], pi, prev[2])
                            pi += 1
                            n -= 1

                    for tk in range(ntk):
                        lo = max(0, 128 * tk - tq0)
                        ps_s = apsum.tile([128, HW], f32, tag="s", bufs=2)
                        # the last half-0 block has no filler: compute the
                        # masked (junk) columns too so PE/HAM stay warm
                        slo = 0 if (half == 0 and not fillers) else lo
                        chunks = ([(slo, 512), (512, HW)] if slo < 512
                                  else [(slo, HW)])
                        for (a, bnd) in chunks:
                            nc.tensor.matmul(
                                ps_s[:, a:bnd],
                                kt_sb[:, 128 * tk:128 * (tk + 1)],
                                qt_sb[h][:, tq0 + a:tq0 + bnd],
                                start=True, stop=True)
                        p_t = ppool.tile([128, HW], bf, tag="p", bufs=pbufs)
                        nc.scalar.activation(p_t[:, lo:HW], ps_s[:, lo:HW],
                                             Exp, scale=SCALE)
                        tiles.append(p_t)
                        if tk >= 1 and (ntk == 8 or tk % 2 == 1):
                            emit_y(1)
                        if (fi < len(fillers) and tk >= fill_start
                                and (tk - fill_start) % fill_step == 0):
                            fillers[fi]()
                            fi += 1
                    emit_y(HW // 128)
                    while fi < len(fillers):
                        fillers[fi]()
                        fi += 1
                    return tiles

                def oproj_tile(ti, n, dve_only, tailq=False):
                    tsl = slice(128 * ti, 128 * (ti + 1))
                    nsl = slice(512 * n, 512 * (n + 1))
                    ps_o = apsum.tile([128, 512], f32, tag="b1",
                                      bufs=3, name="ps_o")
                    for h in range(NHL):
                        nc.tensor.matmul(
                            ps_o, yt_sb[h][:, tsl], wo_sb[:, h, nsl],
                            start=(h == 0), stop=(h == NHL - 1))
                    ob = ap2.tile([128, 512], bf, tag="ob", bufs=16)
                    if tailq and ti < 12:
                        # early tail: keep DVE free for the wide
                        # reciprocal/normalize chain
                        nc.scalar.copy(ob, ps_o)
                    elif dve_only or (ti * 4 + n) % 2 == 0:
                        nc.vector.tensor_copy(ob, ps_o)
                    else:
                        nc.scalar.copy(ob, ps_o)
                    # tail output drains on two queues in parallel
                    if tailq and n % 2 == 0:
                        nc.sync.dma_start(o[tsl, nsl], ob)
                    else:
                        nc.gpsimd.dma_start(o[tsl, nsl], ob)

                def oproj(t0, t1, dve_only, tailq=False):
                    for ti in range(t0, t1):
                        for n in range(C // 512):
                            oproj_tile(ti, n, dve_only, tailq)

                # ---- Q h0 upfront (b1 rotation gives a 3-deep pipeline) --
                for j in range(T // 512):
                    q_unit(0, j)

                # ---- half 0, with the next head's Q-proj units as PE
                # ---- filler under the ACT-bound exp stretch
                with tc.tile_pool(name="attn0", bufs=1) as ap0:
                    prev0 = None
                    for h in range(NHL):
                        if h + 1 < NHL:
                            fil = [(lambda hh, jj: lambda: q_unit(hh, jj))(
                                h + 1, j) for j in range(T // 512)]
                        else:
                            fil = []
                        tiles = attn_block(h, 0, prev0, ap0, 16, fil, 1, 2)
                        prev0 = (h, 0, tiles)
                    # last head's half-0 groups, before ap0 closes
                    for il in range(HW // 128):
                        y_group(NHL - 1, 0, il, prev0[2])
                # x chunks + rope scratch are dead: release them so the
                # half-1 attention pool below can reuse the space
                es1.close()

                ap2_cm = tc.tile_pool(name="attn2", bufs=1)
                ap2 = ap2_cm.__enter__()
                # ---- half 1, with O-proj row tiles woven inside each
                # ---- block (row r needs every head's half-0 Y)
                row_fill = ([(0, n) for n in range(4)] +
                            [(1, n) for n in range(4)] +
                            [(2, n) for n in range(4)],
                            [(3, n) for n in range(4)] +
                            [(4, n) for n in range(4)],
                            [(5, n) for n in range(4)] +
                            [(6, n) for n in range(4)],
                            [(7, n) for n in range(4)])
                prev = None
                for h in range(NHL):
                    fil = [(lambda t, n: lambda: oproj_tile(
                        t, n, dve_only=True))(t, n) for (t, n) in row_fill[h]]
                    tiles = attn_block(h, 1, prev, ap2, 34, fil, 2, 1)
                    prev = (h, 1, tiles)
                od = 8
                # tail: the last head's half-1 Y accumulates V-stationary
                # straight into Y^T (PSUM "s" slots, free once scores end),
                # denominator from an all-ones stationary — no DMA-xbar
                # transposes left between the last exp and the final O-proj.
                tiles3 = prev[2]
                ps_yt = apsum.tile([128, HW], f32, tag="s", bufs=2,
                                   name="ps_yt")
                ps_z = apsum.tile([128, HW], f32, tag="s", bufs=2,
                                  name="ps_z")
                rzw = ap2.tile([128, HW], f32, tag="rzw", bufs=1)

                # fp32 psum writes are one-bank (<=512) wide, so accumulate
                # the two 512-col halves as separate groups; the slow wide
                # reciprocals hide under the other half's matmuls / O-proj
                def yz_chunk(lo0, hi, tkmax):
                    for tk in range(tkmax + 1):
                        a = max(lo0, 128 * tk - HW)
                        st, sp = (tk == 0), (tk == tkmax)
                        nc.tensor.matmul(ps_yt[:, a:hi],
                                         vaug_sb[:, tk, 0:HD],
                                         tiles3[tk][:, a:hi],
                                         start=st, stop=sp)
                        nc.tensor.matmul(ps_z[:, a:hi], ones_sb,
                                         tiles3[tk][:, a:hi],
                                         start=st, stop=sp)

                def norm_piece(piece):
                    psl = slice(512 * piece, 512 * (piece + 1))
                    tsl = slice(HW + 512 * piece, HW + 512 * (piece + 1))
                    nc.vector.tensor_mul(yt_sb[NHL - 1][:, tsl],
                                         ps_yt[:, psl], rzw[:, psl])

                yz_chunk(0, 512, 11)
                nc.vector.reciprocal(rzw[:, 0:512], ps_z[:, 0:512])
                yz_chunk(512, HW, NT - 1)
                norm_piece(0)
                oproj(od, od + 4, dve_only=False, tailq=True)
                nc.vector.reciprocal(rzw[:, 512:HW], ps_z[:, 512:HW])
                norm_piece(1)
                oproj(od + 4, NT, dve_only=False, tailq=True)
                ap2_cm.__exit__(None, None, None)

    nc.finalize()
    return nc


def _tables():
    freqs = 1.0 / (10000.0 ** (np.arange(0, HD, 2, dtype=np.float32) / HD))
    t = np.arange(T, dtype=np.float32)
    emb = np.outer(t, freqs)                  # [T, 64]
    cos_t = np.cos(emb).T.astype(np.float32)  # [64, T]
    sin_t = np.sin(emb).T.astype(np.float32)
    cosT = np.ascontiguousarray(np.concatenate([cos_t, cos_t], 0)).astype(BF16)
    # halves swapped: row d holds the factor multiplying t0[(d+64)%128]
    # when writing t2[d ^ 64 half]; see rope_evict
    sinT = np.ascontiguousarray(np.concatenate([sin_t, -sin_t], 0)).astype(BF16)
    return cosT, sinT


def _get_nc():
    if "nc" not in _CACHE:
        _CACHE["nc"] = _build_nc()
    return _CACHE["nc"]


def kernel(x, Wq, Wk, Wv, Wo, _trace=False):
    from concourse.bass_utils import run_bass_kernel_spmd

    x = np.asarray(x, dtype=np.float32)
    cosT, sinT = _tables()

    def chunked(w):
        # [K, m] -> [128, K//128, m] (partition-major, contiguous)
        k, m = w.shape
        return np.ascontiguousarray(
            w.reshape(k // 128, 128, m).transpose(1, 0, 2)).astype(BF16)

    in_maps = []
    for core in range(8):
        b, g = divmod(core, 4)
        in_maps.append({
            "xT": np.ascontiguousarray(x[b].T).astype(BF16),
            "wq": chunked(Wq[:, 512 * g:512 * (g + 1)]),
            "wk": chunked(Wk[:, 128 * g:128 * (g + 1)]),
            "wv": chunked(Wv[:, 128 * g:128 * (g + 1)]),
            "wo": chunked(Wo[512 * g:512 * (g + 1), :]),
            "cosT": cosT,
            "sinT": sinT,
        })

    nc = _get_nc()
    res = run_bass_kernel_spmd(nc, in_maps, list(range(8)), trace=_trace)
    parts = [np.asarray(res.results[c]["o_part"], dtype=np.float32)
             for c in range(8)]
    out = np.empty((B, T, C), dtype=np.float32)
    for b in range(B):
        out[b] = parts[4 * b] + parts[4 * b + 1] + parts[4 * b + 2] + parts[4 * b + 3]
    if _trace:
        return out, res
    return out



# revision 40
# speedup vs baseline: 1.2498x; 1.2498x over previous
"""Block-causal GQA attention on 8 trn2 NeuronCores.

Sharding: core = b*4 + g  (b in {0,1} batch, g in {0..3} kv-head group).
Each core computes, for its batch b and kv group g (4 q-heads, 1 kv head):
    partial_out = softmax_blockcausal(rope(x@Wq_g) @ rope(x@Wk_g)^T) @ (x@Wv_g) @ Wo_g
Host sums the 4 group partials per batch.

Device design (bf16 matmuls, f32 PSUM):
  - Host passes x^T, so Q^T/K^T/V^T come out of projections with d on
    partitions and no on-device transposes; RoPE (sign folded into the sin
    table) happens on DVE during PSUM eviction.  V^T is DMA-xbar-transposed
    into V_aug = [V | ones].
  - Projections run c-chunk-outer in PSUM waves (K+V, Q0+Q1, Q2+Q3) so PE
    work starts as soon as the first x^T chunk lands.
  - Attention per (head, 1024-wide tq half): S^T[tk,tq] = K^T.T @ Q^T,
    exp on ACT (scale=1/sqrt(128); scores are O(1) so no max subtraction),
    then per 128-wide tq tile: [Y|Z][tq,129] += P^T_tile.T @ V_aug
    (P^T stationary, fused softmax denominator in column 128).
    Normalize: rz = 1/Z [tq,1], Y *= rz via per-partition tensor_scalar,
    DMA-transpose Y tile into Y^T[d, tq].  Half-0 score matmuls compute
    the masked (junk) columns too so PE/HAM stay busy under the
    ACT-bound exp stretch.
  - Tail: the last head's half-1 Y accumulates V-stationary straight
    into Y^T psum (Z via an all-ones stationary), so no DMA-xbar
    transpose sits between the last exp and the final O-proj rows.
  - O[t,n] = sum_h Y_h^T.T @ Wo_h accumulated in PSUM over heads;
    bf16 partials out (host sums in f32); evictions alternate DVE/ACT,
    tail output drains on two DMA queues.
"""
import os
import sys
from contextlib import ExitStack

import numpy as np

for _p in ("/opt/trn_rl_repo",):
    if _p not in sys.path and os.path.isdir(_p):
        sys.path.insert(0, _p)

import ml_dtypes

BF16 = ml_dtypes.bfloat16

B = 2
T = 2048
C = 2048
HD = 128
NHL = 4           # q heads per core
NT = T // 128     # 16 query/key tiles
NCH = C // 128    # 16 contraction chunks
HW = T // 2       # tq half width
SCALE = 1.0 / float(np.sqrt(np.float32(HD)))

_CACHE = {}


def _build_nc():
    import concourse.bass as bass
    import concourse.mybir as mybir
    import concourse.tile as tile
    from concourse import bacc

    dt = mybir.dt
    f32 = dt.float32
    bf = dt.bfloat16
    Exp = mybir.ActivationFunctionType.Exp

    nc = bacc.Bacc(None, target_bir_lowering=False)

    # weights host-prelaid as [partition, chunk, m] so each DMA is 128 fat
    # contiguous descriptors instead of 2048 small ones
    xT = nc.declare_dram_parameter("xT", [C, T], bf, isOutput=False)
    wq = nc.declare_dram_parameter("wq", [128, NCH, NHL * HD], bf, isOutput=False)
    wk = nc.declare_dram_parameter("wk", [128, NCH, HD], bf, isOutput=False)
    wv = nc.declare_dram_parameter("wv", [128, NCH, HD], bf, isOutput=False)
    wo = nc.declare_dram_parameter("wo", [128, NHL, C], bf, isOutput=False)
    cosT = nc.declare_dram_parameter("cosT", [HD, T], bf, isOutput=False)
    sinT = nc.declare_dram_parameter("sinT", [HD, T], bf, isOutput=False)
    # bf16 partials (host sums in f32): halves output DMA + drain backlog
    o = nc.declare_dram_parameter("o_part", [T, C], bf, isOutput=True)

    with tile.TileContext(nc) as tc:
        with tc.tile_pool(name="consts", bufs=1) as consts:
            # ---- static SBUF loads (order = DMA priority) ----
            wk_sb = consts.tile([128, NCH, HD], bf, name="wk_sb")
            nc.sync.dma_start(wk_sb, wk[:, :, :])
            wv_sb = consts.tile([128, NCH, HD], bf, name="wv_sb")
            nc.sync.dma_start(wv_sb, wv[:, :, :])

            cos_sb = consts.tile([128, T], bf, name="cos_sb")
            sin_sb = consts.tile([128, T], bf, name="sin_sb")
            wq_sb = consts.tile([128, NCH, NHL * HD], bf, name="wq_sb")
            wo_sb = consts.tile([128, NHL, C], bf, name="wo_sb")

            # V_aug = [V | ones]: col 128 preset to 1, cols 0:128 filled by
            # DMA-transpose from V^T after the V projection.  Rows are 256
            # wide so each tile's dst offset stays 512B-aligned — the DMA
            # xbar transpose corrupts data at unaligned dst offsets.
            vaug_sb = consts.tile([128, NT, 2 * HD], bf, name="vaug_sb")
            nc.vector.memset(vaug_sb[:, :, HD:HD + 1], 1.0)

            # all-ones stationary for the tail softmax denominator
            ones_sb = consts.tile([128, 128], bf, name="ones_sb")
            nc.vector.memset(ones_sb, 1.0)

            # warm the ACT exp table set during phase 1
            dumm = consts.tile([1, 8], f32, name="dumm")
            nc.vector.memset(dumm, 0.0)
            nc.scalar.activation(dumm, dumm, Exp)

            # persistent activations
            kt_sb = consts.tile([128, T], bf, name="kt_sb")
            vt_sb = consts.tile([128, T], bf, name="vt_sb")
            qt_sb = [consts.tile([128, T], bf, name=f"qt{h}") for h in range(NHL)]
            yt_sb = [consts.tile([128, T], bf, name=f"yt{h}") for h in range(NHL)]

            # ============ phase 1: K/V projection (c-outer wave) ==========
            # x chunks + rope scratch live on the RIGHT side so the half-1
            # attention pool (opened later, left) can reuse their space
            es1 = ExitStack()
            xtp = es1.enter_context(
                tc.tile_pool(name="xtp", bufs=1, side="right"))
            proj = es1.enter_context(
                tc.tile_pool(name="proj", bufs=1, side="right"))

            xt_r = xT.rearrange("(n p) t -> n p t", p=128)
            xt_sb = []
            for cch in range(NCH):
                xt_c = xtp.tile([128, T], bf, name=f"xt{cch}")
                nc.sync.dma_start(xt_c, xt_r[cch])
                xt_sb.append(xt_c)
                if cch == 10:
                    # wq arrives just before the Q waves need it
                    nc.sync.dma_start(wq_sb, wq[:, :, :])
                elif cch == 12:
                    # rope tables land before the K eviction needs them
                    nc.sync.dma_start(cos_sb, cosT[:, :])
                    nc.sync.dma_start(sin_sb, sinT[:, :])
            nc.sync.dma_start(wo_sb, wo[:, :, :])

            def rope_evict(ps, jsl, dst):
                # dst[:, jsl] = ps * cos + rot_half(ps) * sin  (bf16).
                # ACT does the PSUM eviction; DVE runs at bf16 2x.
                t0 = proj.tile([128, 512], bf, tag="t0", bufs=3)
                t1 = proj.tile([128, 512], bf, tag="t1", bufs=3)
                t2 = proj.tile([128, 512], bf, tag="t2", bufs=3)
                # sin table halves are pre-swapped on host so each mul
                # reads both SBUF inputs at the same base partition
                # (walrus requires equal SBUF base partitions).
                nc.scalar.copy(t0, ps)
                nc.vector.tensor_mul(t1, t0, cos_sb[:, jsl])
                nc.vector.tensor_mul(t2[0:64], t0[64:128], sin_sb[64:128, jsl])
                nc.vector.tensor_mul(t2[64:128], t0[0:64], sin_sb[0:64, jsl])
                nc.vector.tensor_add(dst[:, jsl], t1, t2)

            with tc.tile_pool(name="proj_psum", bufs=2, space="PSUM") as pp:
                # warm the PE clock (HAM) with throwaway matmuls while the
                # input DMAs stream in; results are never read
                warm_ps = pp.tile([128, 512], f32, tag="pj", bufs=8,
                                  name="warm_ps")
                for _ in range(28):
                    nc.tensor.matmul(warm_ps[0:1, :],
                                     vaug_sb[:, 0, HD:HD + 1],
                                     kt_sb[:, 0:512], start=True, stop=True)

                # -- wave 1: K and V (c-outer so PE starts with first chunk) --
                ps_k = [pp.tile([128, 512], f32, tag="pj", bufs=8,
                                name=f"ps_k{j}") for j in range(4)]
                ps_v = [pp.tile([128, 512], f32, tag="pj", bufs=8,
                                name=f"ps_v{j}") for j in range(4)]
                for cch in range(NCH):
                    st, sp = (cch == 0), (cch == NCH - 1)
                    for j in range(T // 512):
                        jsl = slice(512 * j, 512 * (j + 1))
                        nc.tensor.matmul(ps_k[j], wk_sb[:, cch, :],
                                         xt_sb[cch][:, jsl], start=st, stop=sp)
                        nc.tensor.matmul(ps_v[j], wv_sb[:, cch, :],
                                         xt_sb[cch][:, jsl], start=st, stop=sp)
                for j in range(T // 512):
                    jsl = slice(512 * j, 512 * (j + 1))
                    rope_evict(ps_k[j], jsl, kt_sb)
                    nc.scalar.copy(vt_sb[:, jsl], ps_v[j])
                for i in range(NT):
                    nc.sync.dma_start_transpose(
                        vaug_sb[:, i, 0:HD], vt_sb[:, 128 * i:128 * (i + 1)]
                    )

            # ===== phases 1b+2+3: Q proj woven into half-0 attention, =====
            # ===== O-proj rows woven into half-1 attention ================
            # PSUM: tag "s" (2x2 banks) + tag "b1" (3x1, shared by Q-proj
            # accumulators, [Y|Z] groups and O-proj tiles) = 7 banks.
            with tc.tile_pool(name="attn", bufs=1) as ap, \
                 tc.tile_pool(name="attn_psum", bufs=1, space="PSUM") as apsum:

                def q_unit(h, j):
                    hsl = slice(HD * h, HD * (h + 1))
                    jsl = slice(512 * j, 512 * (j + 1))
                    ps_q = apsum.tile([128, 512], f32, tag="b1", bufs=3,
                                      name=f"ps_q{h}_{j}")
                    for cch in range(NCH):
                        nc.tensor.matmul(
                            ps_q, wq_sb[:, cch, hsl], xt_sb[cch][:, jsl],
                            start=(cch == 0), stop=(cch == NCH - 1))
                    rope_evict(ps_q, jsl, qt_sb[h])

                def y_group(h, half, il, tiles, eng=None):
                    """One [Y|Z] accumulation + normalize + transpose-out."""
                    gi = (HW // 128) * half + il
                    ps_yz = apsum.tile([128, 512], f32, tag="b1", bufs=3,
                                       name="ps_yz")
                    for tk in range(gi + 1):
                        nc.tensor.matmul(
                            ps_yz[:, 0:HD + 1],
                            tiles[tk][:, 128 * il:128 * (il + 1)],
                            vaug_sb[:, tk, 0:HD + 1],
                            start=(tk == 0), stop=(tk == gi))
                    rz = ap.tile([128, 1], f32, tag="rz", bufs=8)
                    nc.vector.reciprocal(rz, ps_yz[:, HD:HD + 1])
                    ysb = ap.tile([128, HD], bf, tag="ysb", bufs=8)
                    nc.vector.tensor_scalar_mul(ysb, ps_yz[:, 0:HD], rz)
                    (eng or nc.sync).dma_start_transpose(
                        yt_sb[h][:, 128 * gi:128 * (gi + 1)], ysb)

                def attn_block(h, half, prev, ppool, pbufs, fillers,
                               fill_start, fill_step, junk=False):
                    """S^T + exp for this tq half.  The previous block's
                    Y-groups plus the filler closures (Q-proj units in half
                    0, O-proj tiles in half 1) are woven between the tk
                    iterations so PE always has queued work while ACT
                    catches up on exps."""
                    tq0 = HW * half
                    ntk = (tq0 + HW) // 128
                    tiles = []
                    pi = 0
                    fi = 0

                    def emit_y(n):
                        nonlocal pi
                        while n > 0 and prev is not None and pi < HW // 128:
                            y_group(prev[0], prev[1], pi, prev[2])
                            pi += 1
                            n -= 1

                    for tk in range(ntk):
                        lo = max(0, 128 * tk - tq0)
                        ps_s = apsum.tile([128, HW], f32, tag="s", bufs=2)
                        # filler-less ACT-bound blocks: compute the masked
                        # (junk) columns too so PE/HAM stay warm
                        slo = 0 if junk else lo
                        chunks = ([(slo, 512), (512, HW)] if slo < 512
                                  else [(slo, HW)])
                        for (a, bnd) in chunks:
                            nc.tensor.matmul(
                                ps_s[:, a:bnd],
                                kt_sb[:, 128 * tk:128 * (tk + 1)],
                                qt_sb[h][:, tq0 + a:tq0 + bnd],
                                start=True, stop=True)
                        p_t = ppool.tile([128, HW], bf, tag="p", bufs=pbufs)
                        nc.scalar.activation(p_t[:, lo:HW], ps_s[:, lo:HW],
                                             Exp, scale=SCALE)
                        tiles.append(p_t)
                        if tk >= 1 and (ntk == 8 or tk % 2 == 1):
                            emit_y(1)
                        if (fi < len(fillers) and tk >= fill_start
                                and (tk - fill_start) % fill_step == 0):
                            fillers[fi]()
                            fi += 1
                    emit_y(HW // 128)
                    while fi < len(fillers):
                        fillers[fi]()
                        fi += 1
                    return tiles

                def oproj_tile(ti, n, dve_only, tailq=False):
                    tsl = slice(128 * ti, 128 * (ti + 1))
                    nsl = slice(512 * n, 512 * (n + 1))
                    ps_o = apsum.tile([128, 512], f32, tag="b1",
                                      bufs=3, name="ps_o")
                    for h in range(NHL):
                        nc.tensor.matmul(
                            ps_o, yt_sb[h][:, tsl], wo_sb[:, h, nsl],
                            start=(h == 0), stop=(h == NHL - 1))
                    ob = ap2.tile([128, 512], bf, tag="ob", bufs=16)
                    if tailq and ti < 12:
                        # early tail: keep DVE free for the wide
                        # reciprocal/normalize chain
                        nc.scalar.copy(ob, ps_o)
                    elif dve_only or (ti * 4 + n) % 2 == 0:
                        nc.vector.tensor_copy(ob, ps_o)
                    else:
                        nc.scalar.copy(ob, ps_o)
                    # tail output drains on two queues in parallel
                    if tailq and n % 2 == 0:
                        nc.sync.dma_start(o[tsl, nsl], ob)
                    else:
                        nc.gpsimd.dma_start(o[tsl, nsl], ob)

                def oproj(t0, t1, dve_only, tailq=False):
                    for ti in range(t0, t1):
                        for n in range(C // 512):
                            oproj_tile(ti, n, dve_only, tailq)

                # ---- Q h0 upfront (b1 rotation gives a 3-deep pipeline) --
                for j in range(T // 512):
                    q_unit(0, j)

                # ---- half 0, with the next head's Q-proj units as PE
                # ---- filler under the ACT-bound exp stretch
                with tc.tile_pool(name="attn0", bufs=1) as ap0:
                    prev0 = None
                    for h in range(NHL):
                        if h + 1 < NHL:
                            fil = [(lambda hh, jj: lambda: q_unit(hh, jj))(
                                h + 1, j) for j in range(T // 512)]
                        else:
                            fil = []
                        tiles = attn_block(h, 0, prev0, ap0, 16, fil, 1, 2,
                                           junk=not fil)
                        prev0 = (h, 0, tiles)
                    # last head's half-0 groups before ap0 closes;
                    # transposes split over two hwdge queues so the last
                    # lands ~5us sooner for the half-1 O-proj weave
                    for il in range(HW // 128):
                        y_group(NHL - 1, 0, il, prev0[2],
                                eng=(nc.sync, nc.scalar)[il % 2])
                # x chunks + rope scratch are dead: release them so the
                # half-1 attention pool below can reuse the space
                es1.close()

                ap2_cm = tc.tile_pool(name="attn2", bufs=1)
                ap2 = ap2_cm.__enter__()
                # ---- half 1: head 0 (no prev Y-groups to weave) gets two
                # O-proj rows woven inside + junk columns; heads 1-3 weave
                # the previous head's Y-groups with O-proj rows between
                # blocks, as measured-dense
                fil = [(lambda t, n: lambda: oproj_tile(
                    t, n, dve_only=True))(t, n)
                    for t in (0, 1) for n in range(4)]
                tiles = attn_block(0, 1, None, ap2, 34, fil, 4, 1, junk=True)
                prev = (0, 1, tiles)
                od = 2
                for h in range(1, NHL):
                    tiles = attn_block(h, 1, prev, ap2, 34, [], 0, 1)
                    oproj(od, od + 2, dve_only=True)
                    od += 2
                    prev = (h, 1, tiles)
                # tail: the last head's half-1 Y accumulates V-stationary
                # straight into Y^T (PSUM "s" slots, free once scores end),
                # denominator from an all-ones stationary — no DMA-xbar
                # transposes left between the last exp and the final O-proj.
                tiles3 = prev[2]
                ps_yt = apsum.tile([128, HW], f32, tag="s", bufs=2,
                                   name="ps_yt")
                ps_z = apsum.tile([128, HW], f32, tag="s", bufs=2,
                                  name="ps_z")
                rzw = ap2.tile([128, HW], f32, tag="rzw", bufs=1)

                # fp32 psum writes are one-bank (<=512) wide, so accumulate
                # the two 512-col halves as separate groups; the slow wide
                # reciprocals hide under the other half's matmuls / O-proj
                def yz_chunk(lo0, hi, tkmax):
                    for tk in range(tkmax + 1):
                        a = max(lo0, 128 * tk - HW)
                        st, sp = (tk == 0), (tk == tkmax)
                        nc.tensor.matmul(ps_yt[:, a:hi],
                                         vaug_sb[:, tk, 0:HD],
                                         tiles3[tk][:, a:hi],
                                         start=st, stop=sp)
                        nc.tensor.matmul(ps_z[:, a:hi], ones_sb,
                                         tiles3[tk][:, a:hi],
                                         start=st, stop=sp)

                def norm_piece(piece):
                    psl = slice(512 * piece, 512 * (piece + 1))
                    tsl = slice(HW + 512 * piece, HW + 512 * (piece + 1))
                    nc.vector.tensor_mul(yt_sb[NHL - 1][:, tsl],
                                         ps_yt[:, psl], rzw[:, psl])

                yz_chunk(0, 512, 11)
                nc.vector.reciprocal(rzw[:, 0:512], ps_z[:, 0:512])
                yz_chunk(512, HW, NT - 1)
                norm_piece(0)
                oproj(od, od + 4, dve_only=False, tailq=True)
                nc.vector.reciprocal(rzw[:, 512:HW], ps_z[:, 512:HW])
                norm_piece(1)
                oproj(od + 4, NT, dve_only=False, tailq=True)
                ap2_cm.__exit__(None, None, None)

    nc.finalize()
    return nc


def _tables():
    freqs = 1.0 / (10000.0 ** (np.arange(0, HD, 2, dtype=np.float32) / HD))
    t = np.arange(T, dtype=np.float32)
    emb = np.outer(t, freqs)                  # [T, 64]
    cos_t = np.cos(emb).T.astype(np.float32)  # [64, T]
    sin_t = np.sin(emb).T.astype(np.float32)
    cosT = np.ascontiguousarray(np.concatenate([cos_t, cos_t], 0)).astype(BF16)
    # halves swapped: row d holds the factor multiplying t0[(d+64)%128]
    # when writing t2[d ^ 64 half]; see rope_evict
    sinT = np.ascontiguousarray(np.concatenate([sin_t, -sin_t], 0)).astype(BF16)
    return cosT, sinT


def _get_nc():
    if "nc" not in _CACHE:
        _CACHE["nc"] = _build_nc()
    return _CACHE["nc"]


def kernel(x, Wq, Wk, Wv, Wo, _trace=False):
    from concourse.bass_utils import run_bass_kernel_spmd

    x = np.asarray(x, dtype=np.float32)
    cosT, sinT = _tables()

    def chunked(w):
        # [K, m] -> [128, K//128, m] (partition-major, contiguous)
        k, m = w.shape
        return np.ascontiguousarray(
            w.reshape(k // 128, 128, m).transpose(1, 0, 2)).astype(BF16)

    in_maps = []
    for core in range(8):
        b, g = divmod(core, 4)
        in_maps.append({
            "xT": np.ascontiguousarray(x[b].T).astype(BF16),
            "wq": chunked(Wq[:, 512 * g:512 * (g + 1)]),
            "wk": chunked(Wk[:, 128 * g:128 * (g + 1)]),
            "wv": chunked(Wv[:, 128 * g:128 * (g + 1)]),
            "wo": chunked(Wo[512 * g:512 * (g + 1), :]),
            "cosT": cosT,
            "sinT": sinT,
        })

    nc = _get_nc()
    res = run_bass_kernel_spmd(nc, in_maps, list(range(8)), trace=_trace)
    parts = [np.asarray(res.results[c]["o_part"], dtype=np.float32)
             for c in range(8)]
    out = np.empty((B, T, C), dtype=np.float32)
    for b in range(B):
        out[b] = parts[4 * b] + parts[4 * b + 1] + parts[4 * b + 2] + parts[4 * b + 3]
    if _trace:
        return out, res
    return out



# revision 42
# speedup vs baseline: 1.2563x; 1.0052x over previous
"""Block-causal GQA attention on 8 trn2 NeuronCores.

Sharding: core = b*4 + g  (b in {0,1} batch, g in {0..3} kv-head group).
Each core computes, for its batch b and kv group g (4 q-heads, 1 kv head):
    partial_out = softmax_blockcausal(rope(x@Wq_g) @ rope(x@Wk_g)^T) @ (x@Wv_g) @ Wo_g
Host sums the 4 group partials per batch.

Device design (bf16 matmuls, f32 PSUM):
  - Host passes x^T, so Q^T/K^T/V^T come out of projections with d on
    partitions and no on-device transposes; RoPE (sign folded into the sin
    table) happens on DVE during PSUM eviction.  V^T is DMA-xbar-transposed
    into V_aug = [V | ones].
  - Projections run c-chunk-outer in PSUM waves (K+V, Q0+Q1, Q2+Q3) so PE
    work starts as soon as the first x^T chunk lands.
  - Attention per (head, 1024-wide tq half): S^T[tk,tq] = K^T.T @ Q^T,
    exp on ACT (scale=1/sqrt(128); scores are O(1) so no max subtraction),
    then per 128-wide tq tile: [Y|Z][tq,129] += P^T_tile.T @ V_aug
    (P^T stationary, fused softmax denominator in column 128).
    Normalize: rz = 1/Z [tq,1], Y *= rz via per-partition tensor_scalar,
    DMA-transpose Y tile into Y^T[d, tq].  Half-0 score matmuls compute
    the masked (junk) columns too so PE/HAM stay busy under the
    ACT-bound exp stretch.
  - Tail: the last head's half-1 Y accumulates V-stationary straight
    into Y^T psum (Z via an all-ones stationary), so no DMA-xbar
    transpose sits between the last exp and the final O-proj rows.
  - O[t,n] = sum_h Y_h^T.T @ Wo_h accumulated in PSUM over heads;
    bf16 partials out (host sums in f32); evictions alternate DVE/ACT,
    tail output drains on two DMA queues.
"""
import os
import sys
from contextlib import ExitStack

import numpy as np

for _p in ("/opt/trn_rl_repo",):
    if _p not in sys.path and os.path.isdir(_p):
        sys.path.insert(0, _p)

import ml_dtypes

BF16 = ml_dtypes.bfloat16

B = 2
T = 2048
C = 2048
HD = 128
NHL = 4           # q heads per core
NT = T // 128     # 16 query/key tiles
NCH = C // 128    # 16 contraction chunks
HW = T // 2       # tq half width
SCALE = 1.0 / float(np.sqrt(np.float32(HD)))

_CACHE = {}


def _build_nc():
    import concourse.bass as bass
    import concourse.mybir as mybir
    import concourse.tile as tile
    from concourse import bacc

    dt = mybir.dt
    f32 = dt.float32
    bf = dt.bfloat16
    Exp = mybir.ActivationFunctionType.Exp

    nc = bacc.Bacc(None, target_bir_lowering=False)

    # weights host-prelaid as [partition, chunk, m] so each DMA is 128 fat
    # contiguous descriptors instead of 2048 small ones
    xT = nc.declare_dram_parameter("xT", [C, T], bf, isOutput=False)
    wq = nc.declare_dram_parameter("wq", [128, NCH, NHL * HD], bf, isOutput=False)
    wk = nc.declare_dram_parameter("wk", [128, NCH, HD], bf, isOutput=False)
    wv = nc.declare_dram_parameter("wv", [128, NCH, HD], bf, isOutput=False)
    wo = nc.declare_dram_parameter("wo", [128, NHL, C], bf, isOutput=False)
    cosT = nc.declare_dram_parameter("cosT", [HD, T], bf, isOutput=False)
    sinT = nc.declare_dram_parameter("sinT", [HD, T], bf, isOutput=False)
    # bf16 partials (host sums in f32): halves output DMA + drain backlog
    o = nc.declare_dram_parameter("o_part", [T, C], bf, isOutput=True)

    with tile.TileContext(nc) as tc:
        with tc.tile_pool(name="consts", bufs=1) as consts:
            # ---- static SBUF loads (order = DMA priority) ----
            wk_sb = consts.tile([128, NCH, HD], bf, name="wk_sb")
            nc.sync.dma_start(wk_sb, wk[:, :, :])
            wv_sb = consts.tile([128, NCH, HD], bf, name="wv_sb")
            nc.sync.dma_start(wv_sb, wv[:, :, :])

            cos_sb = consts.tile([128, T], bf, name="cos_sb")
            sin_sb = consts.tile([128, T], bf, name="sin_sb")
            wq_sb = consts.tile([128, NCH, NHL * HD], bf, name="wq_sb")
            wo_sb = consts.tile([128, NHL, C], bf, name="wo_sb")

            # V_aug = [V | ones]: col 128 preset to 1, cols 0:128 filled by
            # DMA-transpose from V^T after the V projection.  Rows are 256
            # wide so each tile's dst offset stays 512B-aligned — the DMA
            # xbar transpose corrupts data at unaligned dst offsets.
            vaug_sb = consts.tile([128, NT, 2 * HD], bf, name="vaug_sb")
            nc.vector.memset(vaug_sb[:, :, HD:HD + 1], 1.0)

            # all-ones stationary for the tail softmax denominator
            ones_sb = consts.tile([128, 128], bf, name="ones_sb")
            nc.vector.memset(ones_sb, 1.0)

            # warm the ACT exp table set during phase 1
            dumm = consts.tile([1, 8], f32, name="dumm")
            nc.vector.memset(dumm, 0.0)
            nc.scalar.activation(dumm, dumm, Exp)

            # persistent activations
            kt_sb = consts.tile([128, T], bf, name="kt_sb")
            vt_sb = consts.tile([128, T], bf, name="vt_sb")
            qt_sb = [consts.tile([128, T], bf, name=f"qt{h}") for h in range(NHL)]
            yt_sb = [consts.tile([128, T], bf, name=f"yt{h}") for h in range(NHL)]

            # ============ phase 1: K/V projection (c-outer wave) ==========
            # x chunks + rope scratch live on the RIGHT side so the half-1
            # attention pool (opened later, left) can reuse their space
            es1 = ExitStack()
            xtp = es1.enter_context(
                tc.tile_pool(name="xtp", bufs=1, side="right"))
            proj = es1.enter_context(
                tc.tile_pool(name="proj", bufs=1, side="right"))

            xt_r = xT.rearrange("(n p) t -> n p t", p=128)
            xt_sb = []
            for cch in range(NCH):
                xt_c = xtp.tile([128, T], bf, name=f"xt{cch}")
                nc.sync.dma_start(xt_c, xt_r[cch])
                xt_sb.append(xt_c)
                if cch == 10:
                    # wq arrives just before the Q waves need it
                    nc.sync.dma_start(wq_sb, wq[:, :, :])
                elif cch == 12:
                    # rope tables land before the K eviction needs them
                    nc.sync.dma_start(cos_sb, cosT[:, :])
                    nc.sync.dma_start(sin_sb, sinT[:, :])
            nc.sync.dma_start(wo_sb, wo[:, :, :])

            def rope_evict(ps, jsl, dst):
                # dst[:, jsl] = ps * cos + rot_half(ps) * sin  (bf16).
                # ACT does the PSUM eviction; DVE runs at bf16 2x.
                t0 = proj.tile([128, 512], bf, tag="t0", bufs=3)
                t1 = proj.tile([128, 512], bf, tag="t1", bufs=3)
                t2 = proj.tile([128, 512], bf, tag="t2", bufs=3)
                # sin table halves are pre-swapped on host so each mul
                # reads both SBUF inputs at the same base partition
                # (walrus requires equal SBUF base partitions).
                nc.scalar.copy(t0, ps)
                nc.vector.tensor_mul(t1, t0, cos_sb[:, jsl])
                nc.vector.tensor_mul(t2[0:64], t0[64:128], sin_sb[64:128, jsl])
                nc.vector.tensor_mul(t2[64:128], t0[0:64], sin_sb[0:64, jsl])
                nc.vector.tensor_add(dst[:, jsl], t1, t2)

            with tc.tile_pool(name="proj_psum", bufs=2, space="PSUM") as pp:
                # warm the PE clock (HAM) with throwaway matmuls while the
                # input DMAs stream in; results are never read
                warm_ps = pp.tile([128, 512], f32, tag="pj", bufs=8,
                                  name="warm_ps")
                for _ in range(28):
                    nc.tensor.matmul(warm_ps[0:1, :],
                                     vaug_sb[:, 0, HD:HD + 1],
                                     kt_sb[:, 0:512], start=True, stop=True)

                # -- wave 1: K and V (c-outer so PE starts with first chunk) --
                ps_k = [pp.tile([128, 512], f32, tag="pj", bufs=8,
                                name=f"ps_k{j}") for j in range(4)]
                ps_v = [pp.tile([128, 512], f32, tag="pj", bufs=8,
                                name=f"ps_v{j}") for j in range(4)]
                for cch in range(NCH):
                    st, sp = (cch == 0), (cch == NCH - 1)
                    for j in range(T // 512):
                        jsl = slice(512 * j, 512 * (j + 1))
                        nc.tensor.matmul(ps_k[j], wk_sb[:, cch, :],
                                         xt_sb[cch][:, jsl], start=st, stop=sp)
                        nc.tensor.matmul(ps_v[j], wv_sb[:, cch, :],
                                         xt_sb[cch][:, jsl], start=st, stop=sp)
                for j in range(T // 512):
                    jsl = slice(512 * j, 512 * (j + 1))
                    rope_evict(ps_k[j], jsl, kt_sb)
                    nc.scalar.copy(vt_sb[:, jsl], ps_v[j])
                for i in range(NT):
                    nc.sync.dma_start_transpose(
                        vaug_sb[:, i, 0:HD], vt_sb[:, 128 * i:128 * (i + 1)]
                    )

            # ===== phases 1b+2+3: Q proj woven into half-0 attention, =====
            # ===== O-proj rows woven into half-1 attention ================
            # PSUM: tag "s" (2x2 banks) + tag "b1" (3x1, shared by Q-proj
            # accumulators, [Y|Z] groups and O-proj tiles) = 7 banks.
            with tc.tile_pool(name="attn", bufs=1) as ap, \
                 tc.tile_pool(name="attn_psum", bufs=1, space="PSUM") as apsum:

                def q_unit(h, j):
                    hsl = slice(HD * h, HD * (h + 1))
                    jsl = slice(512 * j, 512 * (j + 1))
                    ps_q = apsum.tile([128, 512], f32, tag="b1", bufs=3,
                                      name=f"ps_q{h}_{j}")
                    for cch in range(NCH):
                        nc.tensor.matmul(
                            ps_q, wq_sb[:, cch, hsl], xt_sb[cch][:, jsl],
                            start=(cch == 0), stop=(cch == NCH - 1))
                    rope_evict(ps_q, jsl, qt_sb[h])

                def y_group(h, half, il, tiles, eng=None):
                    """One [Y|Z] accumulation + normalize + transpose-out."""
                    gi = (HW // 128) * half + il
                    ps_yz = apsum.tile([128, 512], f32, tag="b1", bufs=3,
                                       name="ps_yz")
                    for tk in range(gi + 1):
                        nc.tensor.matmul(
                            ps_yz[:, 0:HD + 1],
                            tiles[tk][:, 128 * il:128 * (il + 1)],
                            vaug_sb[:, tk, 0:HD + 1],
                            start=(tk == 0), stop=(tk == gi))
                    rz = ap.tile([128, 1], f32, tag="rz", bufs=8)
                    nc.vector.reciprocal(rz, ps_yz[:, HD:HD + 1])
                    ysb = ap.tile([128, HD], bf, tag="ysb", bufs=8)
                    nc.vector.tensor_scalar_mul(ysb, ps_yz[:, 0:HD], rz)
                    (eng or nc.sync).dma_start_transpose(
                        yt_sb[h][:, 128 * gi:128 * (gi + 1)], ysb)

                def attn_block(h, half, prev, ppool, pbufs, fillers,
                               fill_start, fill_step, junk=False):
                    """S^T + exp for this tq half.  The previous block's
                    Y-groups plus the filler closures (Q-proj units in half
                    0, O-proj tiles in half 1) are woven between the tk
                    iterations so PE always has queued work while ACT
                    catches up on exps."""
                    tq0 = HW * half
                    ntk = (tq0 + HW) // 128
                    tiles = []
                    pi = 0
                    fi = 0

                    def emit_y(n):
                        nonlocal pi
                        while n > 0 and prev is not None and pi < HW // 128:
                            y_group(prev[0], prev[1], pi, prev[2])
                            pi += 1
                            n -= 1

                    for tk in range(ntk):
                        lo = max(0, 128 * tk - tq0)
                        ps_s = apsum.tile([128, HW], f32, tag="s", bufs=2)
                        # filler-less ACT-bound blocks: compute the masked
                        # (junk) columns too so PE/HAM stay warm
                        slo = 0 if junk else lo
                        chunks = ([(slo, 512), (512, HW)] if slo < 512
                                  else [(slo, HW)])
                        for (a, bnd) in chunks:
                            nc.tensor.matmul(
                                ps_s[:, a:bnd],
                                kt_sb[:, 128 * tk:128 * (tk + 1)],
                                qt_sb[h][:, tq0 + a:tq0 + bnd],
                                start=True, stop=True)
                        p_t = ppool.tile([128, HW], bf, tag="p", bufs=pbufs)
                        nc.scalar.activation(p_t[:, lo:HW], ps_s[:, lo:HW],
                                             Exp, scale=SCALE)
                        tiles.append(p_t)
                        if tk >= 1 and (ntk == 8 or tk % 2 == 1):
                            emit_y(1)
                        if (fi < len(fillers) and tk >= fill_start
                                and (tk - fill_start) % fill_step == 0):
                            fillers[fi]()
                            fi += 1
                    emit_y(HW // 128)
                    while fi < len(fillers):
                        fillers[fi]()
                        fi += 1
                    return tiles

                def oproj_tile(ti, n, dve_only, tailq=False):
                    tsl = slice(128 * ti, 128 * (ti + 1))
                    nsl = slice(512 * n, 512 * (n + 1))
                    ps_o = apsum.tile([128, 512], f32, tag="b1",
                                      bufs=3, name="ps_o")
                    for h in range(NHL):
                        nc.tensor.matmul(
                            ps_o, yt_sb[h][:, tsl], wo_sb[:, h, nsl],
                            start=(h == 0), stop=(h == NHL - 1))
                    ob = ap2.tile([128, 512], bf, tag="ob", bufs=16)
                    if tailq and ti < 12:
                        # early tail: keep DVE free for the wide
                        # reciprocal/normalize chain
                        nc.scalar.copy(ob, ps_o)
                    elif dve_only or (ti * 4 + n) % 2 == 0:
                        nc.vector.tensor_copy(ob, ps_o)
                    else:
                        nc.scalar.copy(ob, ps_o)
                    # tail output drains on two queues in parallel
                    if tailq and n % 2 == 0:
                        nc.sync.dma_start(o[tsl, nsl], ob)
                    else:
                        nc.gpsimd.dma_start(o[tsl, nsl], ob)

                def oproj(t0, t1, dve_only, tailq=False):
                    for ti in range(t0, t1):
                        for n in range(C // 512):
                            oproj_tile(ti, n, dve_only, tailq)

                # ---- Q h0 j0/j1 upfront: a half-0 block only reads its
                # head's first 1024 query columns, so j2/j3 defer into the
                # weave and attention starts two units sooner
                q_unit(0, 0)
                q_unit(0, 1)

                # ---- half 0: each block weaves the next head's j0/j1
                # (needed by the next block) plus its own head's j2/j3
                # (needed from half 1 on) under the ACT-bound exp stretch
                def qf(hh, jj):
                    return lambda: q_unit(hh, jj)

                with tc.tile_pool(name="attn0", bufs=1) as ap0:
                    prev0 = None
                    for h in range(NHL):
                        fil = ([qf(h + 1, 0), qf(h + 1, 1)]
                               if h + 1 < NHL else [])
                        fil += [qf(h, 2), qf(h, 3)]
                        tiles = attn_block(h, 0, prev0, ap0, 16, fil, 1, 2)
                        prev0 = (h, 0, tiles)
                    # last head's half-0 groups before ap0 closes;
                    # transposes split over two hwdge queues so the last
                    # lands ~5us sooner for the half-1 O-proj weave
                    for il in range(HW // 128):
                        y_group(NHL - 1, 0, il, prev0[2],
                                eng=(nc.sync, nc.scalar)[il % 2])
                # x chunks + rope scratch are dead: release them so the
                # half-1 attention pool below can reuse the space
                es1.close()

                ap2_cm = tc.tile_pool(name="attn2", bufs=1)
                ap2 = ap2_cm.__enter__()
                # ---- half 1: head 0 (no prev Y-groups to weave) gets two
                # O-proj rows woven inside + junk columns; heads 1-3 weave
                # the previous head's Y-groups with O-proj rows between
                # blocks, as measured-dense
                fil = [(lambda t, n: lambda: oproj_tile(
                    t, n, dve_only=True))(t, n)
                    for t in (0, 1) for n in range(4)]
                tiles = attn_block(0, 1, None, ap2, 34, fil, 4, 1, junk=True)
                prev = (0, 1, tiles)
                od = 2
                for h in range(1, NHL):
                    tiles = attn_block(h, 1, prev, ap2, 34, [], 0, 1)
                    oproj(od, od + 2, dve_only=True)
                    od += 2
                    prev = (h, 1, tiles)
                # tail: the last head's half-1 Y accumulates V-stationary
                # straight into Y^T (PSUM "s" slots, free once scores end),
                # denominator from an all-ones stationary — no DMA-xbar
                # transposes left between the last exp and the final O-proj.
                tiles3 = prev[2]
                ps_yt = apsum.tile([128, HW], f32, tag="s", bufs=2,
                                   name="ps_yt")
                ps_z = apsum.tile([128, HW], f32, tag="s", bufs=2,
                                  name="ps_z")
                rzw = ap2.tile([128, HW], f32, tag="rzw", bufs=1)

                # fp32 psum writes are one-bank (<=512) wide, so accumulate
                # the two 512-col halves as separate groups; the slow wide
                # reciprocals hide under the other half's matmuls / O-proj
                def yz_chunk(lo0, hi, tkmax):
                    for tk in range(tkmax + 1):
                        a = max(lo0, 128 * tk - HW)
                        st, sp = (tk == 0), (tk == tkmax)
                        nc.tensor.matmul(ps_yt[:, a:hi],
                                         vaug_sb[:, tk, 0:HD],
                                         tiles3[tk][:, a:hi],
                                         start=st, stop=sp)
                        nc.tensor.matmul(ps_z[:, a:hi], ones_sb,
                                         tiles3[tk][:, a:hi],
                                         start=st, stop=sp)

                def norm_piece(piece):
                    psl = slice(512 * piece, 512 * (piece + 1))
                    tsl = slice(HW + 512 * piece, HW + 512 * (piece + 1))
                    nc.vector.tensor_mul(yt_sb[NHL - 1][:, tsl],
                                         ps_yt[:, psl], rzw[:, psl])

                yz_chunk(0, 512, 11)
                nc.vector.reciprocal(rzw[:, 0:512], ps_z[:, 0:512])
                # normalize piece 0 before any chunk-B write is emitted so
                # it can't pick up a same-tile WAR against them
                norm_piece(0)
                yz_chunk(512, HW, NT - 1)
                oproj(od, od + 4, dve_only=False, tailq=True)
                nc.vector.reciprocal(rzw[:, 512:HW], ps_z[:, 512:HW])
                norm_piece(1)
                oproj(od + 4, NT, dve_only=False, tailq=True)
                ap2_cm.__exit__(None, None, None)

    nc.finalize()
    return nc


def _tables():
    freqs = 1.0 / (10000.0 ** (np.arange(0, HD, 2, dtype=np.float32) / HD))
    t = np.arange(T, dtype=np.float32)
    emb = np.outer(t, freqs)                  # [T, 64]
    cos_t = np.cos(emb).T.astype(np.float32)  # [64, T]
    sin_t = np.sin(emb).T.astype(np.float32)
    cosT = np.ascontiguousarray(np.concatenate([cos_t, cos_t], 0)).astype(BF16)
    # halves swapped: row d holds the factor multiplying t0[(d+64)%128]
    # when writing t2[d ^ 64 half]; see rope_evict
    sinT = np.ascontiguousarray(np.concatenate([sin_t, -sin_t], 0)).astype(BF16)
    return cosT, sinT


def _get_nc():
    if "nc" not in _CACHE:
        _CACHE["nc"] = _build_nc()
    return _CACHE["nc"]


def kernel(x, Wq, Wk, Wv, Wo, _trace=False):
    from concourse.bass_utils import run_bass_kernel_spmd

    x = np.asarray(x, dtype=np.float32)
    cosT, sinT = _tables()

    def chunked(w):
        # [K, m] -> [128, K//128, m] (partition-major, contiguous)
        k, m = w.shape
        return np.ascontiguousarray(
            w.reshape(k // 128, 128, m).transpose(1, 0, 2)).astype(BF16)

    in_maps = []
    for core in range(8):
        b, g = divmod(core, 4)
        in_maps.append({
            "xT": np.ascontiguousarray(x[b].T).astype(BF16),
            "wq": chunked(Wq[:, 512 * g:512 * (g + 1)]),
            "wk": chunked(Wk[:, 128 * g:128 * (g + 1)]),
            "wv": chunked(Wv[:, 128 * g:128 * (g + 1)]),
            "wo": chunked(Wo[512 * g:512 * (g + 1), :]),
            "cosT": cosT,
            "sinT": sinT,
        })

    nc = _get_nc()
    res = run_bass_kernel_spmd(nc, in_maps, list(range(8)), trace=_trace)
    parts = [np.asarray(res.results[c]["o_part"], dtype=np.float32)
             for c in range(8)]
    out = np.empty((B, T, C), dtype=np.float32)
    for b in range(B):
        out[b] = parts[4 * b] + parts[4 * b + 1] + parts[4 * b + 2] + parts[4 * b + 3]
    if _trace:
        return out, res
    return out



# revision 45
# speedup vs baseline: 1.2668x; 1.0083x over previous
"""Block-causal GQA attention on 8 trn2 NeuronCores.

Sharding: core = b*4 + g  (b in {0,1} batch, g in {0..3} kv-head group).
Each core computes, for its batch b and kv group g (4 q-heads, 1 kv head):
    partial_out = softmax_blockcausal(rope(x@Wq_g) @ rope(x@Wk_g)^T) @ (x@Wv_g) @ Wo_g
Host sums the 4 group partials per batch.

Device design (bf16 matmuls, f32 PSUM):
  - Host passes x^T, so Q^T/K^T/V^T come out of projections with d on
    partitions and no on-device transposes; RoPE (sign folded into the sin
    table) happens on DVE during PSUM eviction.  V^T is DMA-xbar-transposed
    into V_aug = [V | ones].
  - Projections run c-chunk-outer in PSUM waves (K+V, Q0+Q1, Q2+Q3) so PE
    work starts as soon as the first x^T chunk lands.
  - Attention per (head, 1024-wide tq half): S^T[tk,tq] = K^T.T @ Q^T,
    exp on ACT (scale=1/sqrt(128); scores are O(1) so no max subtraction),
    then per 128-wide tq tile: [Y|Z][tq,129] += P^T_tile.T @ V_aug
    (P^T stationary, fused softmax denominator in column 128).
    Normalize: rz = 1/Z [tq,1], Y *= rz via per-partition tensor_scalar,
    DMA-transpose Y tile into Y^T[d, tq].  Half-0 score matmuls compute
    the masked (junk) columns too so PE/HAM stay busy under the
    ACT-bound exp stretch.
  - Tail: the last head's half-1 Y accumulates V-stationary straight
    into Y^T psum (Z via an all-ones stationary), so no DMA-xbar
    transpose sits between the last exp and the final O-proj rows.
  - O[t,n] = sum_h Y_h^T.T @ Wo_h accumulated in PSUM over heads;
    bf16 partials out (host sums in f32); evictions alternate DVE/ACT,
    tail output drains on two DMA queues.
"""
import os
import sys
from contextlib import ExitStack

import numpy as np

for _p in ("/opt/trn_rl_repo",):
    if _p not in sys.path and os.path.isdir(_p):
        sys.path.insert(0, _p)

import ml_dtypes

BF16 = ml_dtypes.bfloat16

B = 2
T = 2048
C = 2048
HD = 128
NHL = 4           # q heads per core
NT = T // 128     # 16 query/key tiles
NCH = C // 128    # 16 contraction chunks
HW = T // 2       # tq half width
SCALE = 1.0 / float(np.sqrt(np.float32(HD)))

_CACHE = {}


def _build_nc():
    import concourse.bass as bass
    import concourse.mybir as mybir
    import concourse.tile as tile
    from concourse import bacc

    dt = mybir.dt
    f32 = dt.float32
    bf = dt.bfloat16
    Exp = mybir.ActivationFunctionType.Exp

    nc = bacc.Bacc(None, target_bir_lowering=False)

    # weights host-prelaid as [partition, chunk, m] so each DMA is 128 fat
    # contiguous descriptors instead of 2048 small ones
    xT = nc.declare_dram_parameter("xT", [C, T], bf, isOutput=False)
    wq = nc.declare_dram_parameter("wq", [128, NCH, NHL * HD], bf, isOutput=False)
    wk = nc.declare_dram_parameter("wk", [128, NCH, HD], bf, isOutput=False)
    wv = nc.declare_dram_parameter("wv", [128, NCH, HD], bf, isOutput=False)
    wo = nc.declare_dram_parameter("wo", [128, NHL, C], bf, isOutput=False)
    cosT = nc.declare_dram_parameter("cosT", [HD, T], bf, isOutput=False)
    sinT = nc.declare_dram_parameter("sinT", [HD, T], bf, isOutput=False)
    # bf16 partials (host sums in f32): halves output DMA + drain backlog
    o = nc.declare_dram_parameter("o_part", [T, C], bf, isOutput=True)

    with tile.TileContext(nc) as tc:
        with tc.tile_pool(name="consts", bufs=1) as consts:
            # ---- static SBUF loads (order = DMA priority) ----
            wk_sb = consts.tile([128, NCH, HD], bf, name="wk_sb")
            nc.sync.dma_start(wk_sb, wk[:, :, :])
            wv_sb = consts.tile([128, NCH, HD], bf, name="wv_sb")
            nc.sync.dma_start(wv_sb, wv[:, :, :])

            cos_sb = consts.tile([128, T], bf, name="cos_sb")
            sin_sb = consts.tile([128, T], bf, name="sin_sb")
            wq_sb = consts.tile([128, NCH, NHL * HD], bf, name="wq_sb")
            wo_sb = consts.tile([128, NHL, C], bf, name="wo_sb")

            # V_aug = [V | ones]: col 128 preset to 1, cols 0:128 filled by
            # DMA-transpose from V^T after the V projection.  Rows are 256
            # wide so each tile's dst offset stays 512B-aligned — the DMA
            # xbar transpose corrupts data at unaligned dst offsets.
            vaug_sb = consts.tile([128, NT, 2 * HD], bf, name="vaug_sb")
            nc.vector.memset(vaug_sb[:, :, HD:HD + 1], 1.0)

            # all-ones stationary for the tail softmax denominator
            ones_sb = consts.tile([128, 128], bf, name="ones_sb")
            nc.vector.memset(ones_sb, 1.0)

            # warm the ACT exp table set during phase 1
            dumm = consts.tile([1, 8], f32, name="dumm")
            nc.vector.memset(dumm, 0.0)
            nc.scalar.activation(dumm, dumm, Exp)

            # persistent activations
            kt_sb = consts.tile([128, T], bf, name="kt_sb")
            vt_sb = consts.tile([128, T], bf, name="vt_sb")
            qt_sb = [consts.tile([128, T], bf, name=f"qt{h}") for h in range(NHL)]
            yt_sb = [consts.tile([128, T], bf, name=f"yt{h}") for h in range(NHL)]

            # ============ phase 1: K/V projection (c-outer wave) ==========
            # x chunks + rope scratch live on the RIGHT side so the half-1
            # attention pool (opened later, left) can reuse their space
            es1 = ExitStack()
            xtp = es1.enter_context(
                tc.tile_pool(name="xtp", bufs=1, side="right"))
            proj = es1.enter_context(
                tc.tile_pool(name="proj", bufs=1, side="right"))

            xt_r = xT.rearrange("(n p) t -> n p t", p=128)
            xt_sb = []
            for cch in range(NCH):
                xt_c = xtp.tile([128, T], bf, name=f"xt{cch}")
                nc.sync.dma_start(xt_c, xt_r[cch])
                xt_sb.append(xt_c)
                if cch == 10:
                    # wq arrives just before the Q waves need it
                    nc.sync.dma_start(wq_sb, wq[:, :, :])
                elif cch == 12:
                    # rope tables land before the K eviction needs them
                    nc.sync.dma_start(cos_sb, cosT[:, :])
                    nc.sync.dma_start(sin_sb, sinT[:, :])
            nc.sync.dma_start(wo_sb, wo[:, :, :])

            def rope_evict(ps, jsl, dst):
                # dst[:, jsl] = ps * cos + rot_half(ps) * sin  (bf16).
                # ACT does the PSUM eviction; DVE runs at bf16 2x.
                t0 = proj.tile([128, 512], bf, tag="t0", bufs=3)
                t1 = proj.tile([128, 512], bf, tag="t1", bufs=3)
                t2 = proj.tile([128, 512], bf, tag="t2", bufs=3)
                # sin table halves are pre-swapped on host so each mul
                # reads both SBUF inputs at the same base partition
                # (walrus requires equal SBUF base partitions).
                nc.scalar.copy(t0, ps)
                nc.vector.tensor_mul(t1, t0, cos_sb[:, jsl])
                nc.vector.tensor_mul(t2[0:64], t0[64:128], sin_sb[64:128, jsl])
                nc.vector.tensor_mul(t2[64:128], t0[0:64], sin_sb[0:64, jsl])
                nc.vector.tensor_add(dst[:, jsl], t1, t2)

            with tc.tile_pool(name="proj_psum", bufs=2, space="PSUM") as pp:
                # warm the PE clock (HAM) with throwaway matmuls while the
                # input DMAs stream in; results are never read
                warm_ps = pp.tile([128, 512], f32, tag="pj", bufs=8,
                                  name="warm_ps")
                for _ in range(28):
                    nc.tensor.matmul(warm_ps[0:1, :],
                                     vaug_sb[:, 0, HD:HD + 1],
                                     kt_sb[:, 0:512], start=True, stop=True)

                # -- wave 1: K and V (c-outer so PE starts with first chunk) --
                ps_k = [pp.tile([128, 512], f32, tag="pj", bufs=8,
                                name=f"ps_k{j}") for j in range(4)]
                ps_v = [pp.tile([128, 512], f32, tag="pj", bufs=8,
                                name=f"ps_v{j}") for j in range(4)]
                for cch in range(NCH):
                    st, sp = (cch == 0), (cch == NCH - 1)
                    for j in range(T // 512):
                        jsl = slice(512 * j, 512 * (j + 1))
                        nc.tensor.matmul(ps_k[j], wk_sb[:, cch, :],
                                         xt_sb[cch][:, jsl], start=st, stop=sp)
                        nc.tensor.matmul(ps_v[j], wv_sb[:, cch, :],
                                         xt_sb[cch][:, jsl], start=st, stop=sp)
                for j in range(T // 512):
                    jsl = slice(512 * j, 512 * (j + 1))
                    rope_evict(ps_k[j], jsl, kt_sb)
                    nc.scalar.copy(vt_sb[:, jsl], ps_v[j])
                for i in range(NT):
                    nc.sync.dma_start_transpose(
                        vaug_sb[:, i, 0:HD], vt_sb[:, 128 * i:128 * (i + 1)]
                    )

            # ===== phases 1b+2+3: Q proj woven into half-0 attention, =====
            # ===== O-proj rows woven into half-1 attention ================
            # PSUM: tag "s" (2x2 banks) + tag "b1" (3x1, shared by Q-proj
            # accumulators, [Y|Z] groups and O-proj tiles) = 7 banks.
            with tc.tile_pool(name="attn", bufs=1) as ap, \
                 tc.tile_pool(name="attn_psum", bufs=1, space="PSUM") as apsum:

                def q_unit(h, j):
                    hsl = slice(HD * h, HD * (h + 1))
                    jsl = slice(512 * j, 512 * (j + 1))
                    ps_q = apsum.tile([128, 512], f32, tag="b1", bufs=3,
                                      name=f"ps_q{h}_{j}")
                    for cch in range(NCH):
                        nc.tensor.matmul(
                            ps_q, wq_sb[:, cch, hsl], xt_sb[cch][:, jsl],
                            start=(cch == 0), stop=(cch == NCH - 1))
                    rope_evict(ps_q, jsl, qt_sb[h])

                def y_group(h, half, il, tiles, eng=None):
                    """One [Y|Z] accumulation + normalize + transpose-out."""
                    gi = (HW // 128) * half + il
                    ps_yz = apsum.tile([128, 512], f32, tag="b1", bufs=3,
                                       name="ps_yz")
                    for tk in range(gi + 1):
                        nc.tensor.matmul(
                            ps_yz[:, 0:HD + 1],
                            tiles[tk][:, 128 * il:128 * (il + 1)],
                            vaug_sb[:, tk, 0:HD + 1],
                            start=(tk == 0), stop=(tk == gi))
                    rz = ap.tile([128, 1], f32, tag="rz", bufs=8)
                    nc.vector.reciprocal(rz, ps_yz[:, HD:HD + 1])
                    ysb = ap.tile([128, HD], bf, tag="ysb", bufs=8)
                    nc.vector.tensor_scalar_mul(ysb, ps_yz[:, 0:HD], rz)
                    (eng or nc.sync).dma_start_transpose(
                        yt_sb[h][:, 128 * gi:128 * (gi + 1)], ysb)

                def attn_block(h, half, prev, ppool, pbufs, fillers,
                               fill_start, fill_step, junk=False,
                               self_y=False):
                    """S^T + exp for this tq half.  The previous block's
                    Y-groups plus the filler closures (Q-proj units in half
                    0, O-proj tiles in half 1) are woven between the tk
                    iterations so PE always has queued work while ACT
                    catches up on exps."""
                    tq0 = HW * half
                    ntk = (tq0 + HW) // 128
                    tiles = []
                    pi = 0
                    fi = 0

                    def emit_y(n):
                        nonlocal pi
                        while n > 0 and prev is not None and pi < HW // 128:
                            y_group(prev[0], prev[1], pi, prev[2])
                            pi += 1
                            n -= 1

                    for tk in range(ntk):
                        lo = max(0, 128 * tk - tq0)
                        ps_s = apsum.tile([128, HW], f32, tag="s", bufs=2)
                        # filler-less ACT-bound blocks: compute the masked
                        # (junk) columns too so PE/HAM stay warm
                        slo = 0 if junk else lo
                        chunks = ([(slo, 512), (512, HW)] if slo < 512
                                  else [(slo, HW)])
                        for (a, bnd) in chunks:
                            nc.tensor.matmul(
                                ps_s[:, a:bnd],
                                kt_sb[:, 128 * tk:128 * (tk + 1)],
                                qt_sb[h][:, tq0 + a:tq0 + bnd],
                                start=True, stop=True)
                        p_t = ppool.tile([128, HW], bf, tag="p", bufs=pbufs)
                        nc.scalar.activation(p_t[:, lo:HW], ps_s[:, lo:HW],
                                             Exp, scale=SCALE)
                        tiles.append(p_t)
                        if tk >= 1 and (ntk == 8 or tk % 2 == 1):
                            emit_y(1)
                        if (fi < len(fillers) and tk >= fill_start
                                and (tk - fill_start) % fill_step == 0):
                            fillers[fi]()
                            fi += 1
                        if self_y and tk >= 2:
                            # last half-0 block: emit its own Y-groups as
                            # exps complete so the transposes finish
                            # before half 1 needs them
                            y_group(h, half, tk - 2, tiles,
                                    eng=(nc.sync, nc.scalar)[tk % 2])
                    emit_y(HW // 128)
                    while fi < len(fillers):
                        fillers[fi]()
                        fi += 1
                    return tiles

                def oproj_tile(ti, n, dve_only, tailq=False):
                    tsl = slice(128 * ti, 128 * (ti + 1))
                    nsl = slice(512 * n, 512 * (n + 1))
                    ps_o = apsum.tile([128, 512], f32, tag="b1",
                                      bufs=3, name="ps_o")
                    for h in range(NHL):
                        nc.tensor.matmul(
                            ps_o, yt_sb[h][:, tsl], wo_sb[:, h, nsl],
                            start=(h == 0), stop=(h == NHL - 1))
                    ob = ap2.tile([128, 512], bf, tag="ob", bufs=16)
                    if tailq and ti < 12:
                        # early tail: keep DVE free for the wide
                        # reciprocal/normalize chain
                        nc.scalar.copy(ob, ps_o)
                    elif dve_only or (ti * 4 + n) % 2 == 0:
                        nc.vector.tensor_copy(ob, ps_o)
                    else:
                        nc.scalar.copy(ob, ps_o)
                    # tail output drains on two queues in parallel
                    if tailq and n % 2 == 0:
                        nc.sync.dma_start(o[tsl, nsl], ob)
                    else:
                        nc.gpsimd.dma_start(o[tsl, nsl], ob)

                def oproj(t0, t1, dve_only, tailq=False):
                    for ti in range(t0, t1):
                        for n in range(C // 512):
                            oproj_tile(ti, n, dve_only, tailq)

                # ---- Q h0 j0/j1 upfront: a half-0 block only reads its
                # head's first 1024 query columns, so j2/j3 defer into the
                # weave and attention starts two units sooner
                q_unit(0, 0)
                q_unit(0, 1)

                # ---- half 0: each block weaves the next head's j0/j1
                # (needed by the next block) plus its own head's j2/j3
                # (needed from half 1 on) under the ACT-bound exp stretch
                def qf(hh, jj):
                    return lambda: q_unit(hh, jj)

                with tc.tile_pool(name="attn0", bufs=1) as ap0:
                    prev0 = None
                    for h in range(NHL):
                        fil = ([qf(h + 1, 0), qf(h + 1, 1)]
                               if h + 1 < NHL else [])
                        fil += [qf(h, 2), qf(h, 3)]
                        tiles = attn_block(h, 0, prev0, ap0, 16, fil, 1, 2,
                                           self_y=(h == NHL - 1))
                        prev0 = (h, 0, tiles)
                    # groups il 0..5 self-wove into the last block above
                    for il in range(6, HW // 128):
                        y_group(NHL - 1, 0, il, prev0[2],
                                eng=(nc.sync, nc.scalar)[il % 2])
                # x chunks + rope scratch are dead: release them so the
                # half-1 attention pool below can reuse the space
                es1.close()

                ap2_cm = tc.tile_pool(name="attn2", bufs=1)
                ap2 = ap2_cm.__enter__()
                # ---- half 1: head 0 (no prev Y-groups to weave) gets two
                # O-proj rows woven inside + junk columns; heads 1-3 weave
                # the previous head's Y-groups with O-proj rows between
                # blocks, as measured-dense
                fil = [(lambda t, n: lambda: oproj_tile(
                    t, n, dve_only=True))(t, n)
                    for t in (0, 1) for n in range(4)]
                tiles = attn_block(0, 1, None, ap2, 34, fil, 4, 1, junk=True)
                prev = (0, 1, tiles)
                od = 2
                for h in range(1, NHL):
                    tiles = attn_block(h, 1, prev, ap2, 34, [], 0, 1)
                    oproj(od, od + 2, dve_only=True)
                    od += 2
                    prev = (h, 1, tiles)
                # tail: the last head's half-1 Y accumulates V-stationary
                # straight into Y^T (PSUM "s" slots, free once scores end),
                # denominator from an all-ones stationary — no DMA-xbar
                # transposes left between the last exp and the final O-proj.
                tiles3 = prev[2]
                ps_yt = apsum.tile([128, HW], f32, tag="s", bufs=2,
                                   name="ps_yt")
                ps_z = apsum.tile([128, HW], f32, tag="s", bufs=2,
                                  name="ps_z")
                rzw = ap2.tile([128, HW], f32, tag="rzw", bufs=1)

                # fp32 psum writes are one-bank (<=512) wide, so accumulate
                # the two 512-col halves as separate groups; the slow wide
                # reciprocals hide under the other half's matmuls / O-proj
                def yz_chunk(lo0, hi, tkmax):
                    for tk in range(tkmax + 1):
                        a = max(lo0, 128 * tk - HW)
                        st, sp = (tk == 0), (tk == tkmax)
                        nc.tensor.matmul(ps_yt[:, a:hi],
                                         vaug_sb[:, tk, 0:HD],
                                         tiles3[tk][:, a:hi],
                                         start=st, stop=sp)
                        nc.tensor.matmul(ps_z[:, a:hi], ones_sb,
                                         tiles3[tk][:, a:hi],
                                         start=st, stop=sp)

                def norm_piece(piece):
                    psl = slice(512 * piece, 512 * (piece + 1))
                    tsl = slice(HW + 512 * piece, HW + 512 * (piece + 1))
                    nc.vector.tensor_mul(yt_sb[NHL - 1][:, tsl],
                                         ps_yt[:, psl], rzw[:, psl])

                yz_chunk(0, 512, 11)
                nc.vector.reciprocal(rzw[:, 0:512], ps_z[:, 0:512])
                # normalize piece 0 before any chunk-B write is emitted so
                # it can't pick up a same-tile WAR against them
                norm_piece(0)
                yz_chunk(512, HW, NT - 1)
                oproj(od, od + 4, dve_only=False, tailq=True)
                nc.vector.reciprocal(rzw[:, 512:HW], ps_z[:, 512:HW])
                norm_piece(1)
                oproj(od + 4, NT, dve_only=False, tailq=True)
                ap2_cm.__exit__(None, None, None)

    nc.finalize()
    return nc


def _tables():
    freqs = 1.0 / (10000.0 ** (np.arange(0, HD, 2, dtype=np.float32) / HD))
    t = np.arange(T, dtype=np.float32)
    emb = np.outer(t, freqs)                  # [T, 64]
    cos_t = np.cos(emb).T.astype(np.float32)  # [64, T]
    sin_t = np.sin(emb).T.astype(np.float32)
    cosT = np.ascontiguousarray(np.concatenate([cos_t, cos_t], 0)).astype(BF16)
    # halves swapped: row d holds the factor multiplying t0[(d+64)%128]
    # when writing t2[d ^ 64 half]; see rope_evict
    sinT = np.ascontiguousarray(np.concatenate([sin_t, -sin_t], 0)).astype(BF16)
    return cosT, sinT


def _get_nc():
    if "nc" not in _CACHE:
        _CACHE["nc"] = _build_nc()
    return _CACHE["nc"]


def kernel(x, Wq, Wk, Wv, Wo, _trace=False):
    from concourse.bass_utils import run_bass_kernel_spmd

    x = np.asarray(x, dtype=np.float32)
    cosT, sinT = _tables()

    def chunked(w):
        # [K, m] -> [128, K//128, m] (partition-major, contiguous)
        k, m = w.shape
        return np.ascontiguousarray(
            w.reshape(k // 128, 128, m).transpose(1, 0, 2)).astype(BF16)

    in_maps = []
    for core in range(8):
        b, g = divmod(core, 4)
        in_maps.append({
            "xT": np.ascontiguousarray(x[b].T).astype(BF16),
            "wq": chunked(Wq[:, 512 * g:512 * (g + 1)]),
            "wk": chunked(Wk[:, 128 * g:128 * (g + 1)]),
            "wv": chunked(Wv[:, 128 * g:128 * (g + 1)]),
            "wo": chunked(Wo[512 * g:512 * (g + 1), :]),
            "cosT": cosT,
            "sinT": sinT,
        })

    nc = _get_nc()
    res = run_bass_kernel_spmd(nc, in_maps, list(range(8)), trace=_trace)
    parts = [np.asarray(res.results[c]["o_part"], dtype=np.float32)
             for c in range(8)]
    out = np.empty((B, T, C), dtype=np.float32)
    for b in range(B):
        out[b] = parts[4 * b] + parts[4 * b + 1] + parts[4 * b + 2] + parts[4 * b + 3]
    if _trace:
        return out, res
    return out



# revision 47
# speedup vs baseline: 1.2736x; 1.0054x over previous
"""Block-causal GQA attention on 8 trn2 NeuronCores.

Sharding: core = b*4 + g  (b in {0,1} batch, g in {0..3} kv-head group).
Each core computes, for its batch b and kv group g (4 q-heads, 1 kv head):
    partial_out = softmax_blockcausal(rope(x@Wq_g) @ rope(x@Wk_g)^T) @ (x@Wv_g) @ Wo_g
Host sums the 4 group partials per batch.

Device design (bf16 matmuls, f32 PSUM):
  - Host passes x^T, so Q^T/K^T/V^T come out of projections with d on
    partitions and no on-device transposes; RoPE (sign folded into the sin
    table) happens on DVE during PSUM eviction.  V^T is DMA-xbar-transposed
    into V_aug = [V | ones].
  - Projections run c-chunk-outer in PSUM waves (K+V, Q0+Q1, Q2+Q3) so PE
    work starts as soon as the first x^T chunk lands.
  - Attention per (head, 1024-wide tq half): S^T[tk,tq] = K^T.T @ Q^T,
    exp on ACT (scale=1/sqrt(128); scores are O(1) so no max subtraction),
    then per 128-wide tq tile: [Y|Z][tq,129] += P^T_tile.T @ V_aug
    (P^T stationary, fused softmax denominator in column 128).
    Normalize: rz = 1/Z [tq,1], Y *= rz via per-partition tensor_scalar,
    DMA-transpose Y tile into Y^T[d, tq].  Half-0 score matmuls compute
    the masked (junk) columns too so PE/HAM stay busy under the
    ACT-bound exp stretch.
  - Tail: the last head's half-1 Y accumulates V-stationary straight
    into Y^T psum (Z via an all-ones stationary), so no DMA-xbar
    transpose sits between the last exp and the final O-proj rows.
  - O[t,n] = sum_h Y_h^T.T @ Wo_h accumulated in PSUM over heads;
    bf16 partials out (host sums in f32); evictions alternate DVE/ACT,
    tail output drains on two DMA queues.
"""
import os
import sys
from contextlib import ExitStack

import numpy as np

for _p in ("/opt/trn_rl_repo",):
    if _p not in sys.path and os.path.isdir(_p):
        sys.path.insert(0, _p)

import ml_dtypes

BF16 = ml_dtypes.bfloat16

B = 2
T = 2048
C = 2048
HD = 128
NHL = 4           # q heads per core
NT = T // 128     # 16 query/key tiles
NCH = C // 128    # 16 contraction chunks
HW = T // 2       # tq half width
SCALE = 1.0 / float(np.sqrt(np.float32(HD)))

_CACHE = {}


def _build_nc():
    import concourse.bass as bass
    import concourse.mybir as mybir
    import concourse.tile as tile
    from concourse import bacc

    dt = mybir.dt
    f32 = dt.float32
    bf = dt.bfloat16
    Exp = mybir.ActivationFunctionType.Exp

    nc = bacc.Bacc(None, target_bir_lowering=False)

    # weights host-prelaid as [partition, chunk, m] so each DMA is 128 fat
    # contiguous descriptors instead of 2048 small ones
    xT = nc.declare_dram_parameter("xT", [C, T], bf, isOutput=False)
    wq = nc.declare_dram_parameter("wq", [128, NCH, NHL * HD], bf, isOutput=False)
    wk = nc.declare_dram_parameter("wk", [128, NCH, HD], bf, isOutput=False)
    wv = nc.declare_dram_parameter("wv", [128, NCH, HD], bf, isOutput=False)
    wo = nc.declare_dram_parameter("wo", [128, NHL, C], bf, isOutput=False)
    cosT = nc.declare_dram_parameter("cosT", [HD, T], bf, isOutput=False)
    sinT = nc.declare_dram_parameter("sinT", [HD, T], bf, isOutput=False)
    # bf16 partials (host sums in f32): halves output DMA + drain backlog
    o = nc.declare_dram_parameter("o_part", [T, C], bf, isOutput=True)

    with tile.TileContext(nc) as tc:
        with tc.tile_pool(name="consts", bufs=1) as consts:
            # ---- static SBUF loads (order = DMA priority) ----
            wk_sb = consts.tile([128, NCH, HD], bf, name="wk_sb")
            nc.sync.dma_start(wk_sb, wk[:, :, :])
            wv_sb = consts.tile([128, NCH, HD], bf, name="wv_sb")
            nc.sync.dma_start(wv_sb, wv[:, :, :])

            cos_sb = consts.tile([128, T], bf, name="cos_sb")
            sin_sb = consts.tile([128, T], bf, name="sin_sb")
            wq_sb = consts.tile([128, NCH, NHL * HD], bf, name="wq_sb")
            wo_sb = consts.tile([128, NHL, C], bf, name="wo_sb")

            # V_aug = [V | ones]: col 128 preset to 1, cols 0:128 filled by
            # DMA-transpose from V^T after the V projection.  Rows are 256
            # wide so each tile's dst offset stays 512B-aligned — the DMA
            # xbar transpose corrupts data at unaligned dst offsets.
            vaug_sb = consts.tile([128, NT, 2 * HD], bf, name="vaug_sb")
            nc.vector.memset(vaug_sb[:, :, HD:HD + 1], 1.0)

            # all-ones stationary for the tail softmax denominator
            ones_sb = consts.tile([128, 128], bf, name="ones_sb")
            nc.vector.memset(ones_sb, 1.0)

            # warm the ACT exp table set during phase 1
            dumm = consts.tile([1, 8], f32, name="dumm")
            nc.vector.memset(dumm, 0.0)
            nc.scalar.activation(dumm, dumm, Exp)

            # persistent activations
            kt_sb = consts.tile([128, T], bf, name="kt_sb")
            vt_sb = consts.tile([128, T], bf, name="vt_sb")
            qt_sb = [consts.tile([128, T], bf, name=f"qt{h}") for h in range(NHL)]
            yt_sb = [consts.tile([128, T], bf, name=f"yt{h}") for h in range(NHL)]

            # ============ phase 1: K/V projection (c-outer wave) ==========
            # x chunks + rope scratch live on the RIGHT side so the half-1
            # attention pool (opened later, left) can reuse their space
            es1 = ExitStack()
            xtp = es1.enter_context(
                tc.tile_pool(name="xtp", bufs=1, side="right"))
            proj = es1.enter_context(
                tc.tile_pool(name="proj", bufs=1, side="right"))

            xt_r = xT.rearrange("(n p) t -> n p t", p=128)
            xt_sb = []
            for cch in range(NCH):
                xt_c = xtp.tile([128, T], bf, name=f"xt{cch}")
                nc.sync.dma_start(xt_c, xt_r[cch])
                xt_sb.append(xt_c)
                if cch == 10:
                    # wq arrives just before the Q waves need it
                    nc.sync.dma_start(wq_sb, wq[:, :, :])
                elif cch == 12:
                    # rope tables land before the K eviction needs them
                    nc.sync.dma_start(cos_sb, cosT[:, :])
                    nc.sync.dma_start(sin_sb, sinT[:, :])
            nc.sync.dma_start(wo_sb, wo[:, :, :])

            def rope_evict(ps, jsl, dst):
                # dst[:, jsl] = ps * cos + rot_half(ps) * sin  (bf16).
                # ACT does the PSUM eviction; DVE runs at bf16 2x.
                t0 = proj.tile([128, 512], bf, tag="t0", bufs=3)
                t1 = proj.tile([128, 512], bf, tag="t1", bufs=3)
                t2 = proj.tile([128, 512], bf, tag="t2", bufs=3)
                # sin table halves are pre-swapped on host so each mul
                # reads both SBUF inputs at the same base partition
                # (walrus requires equal SBUF base partitions).
                nc.scalar.copy(t0, ps)
                nc.vector.tensor_mul(t1, t0, cos_sb[:, jsl])
                nc.vector.tensor_mul(t2[0:64], t0[64:128], sin_sb[64:128, jsl])
                nc.vector.tensor_mul(t2[64:128], t0[0:64], sin_sb[0:64, jsl])
                nc.vector.tensor_add(dst[:, jsl], t1, t2)

            with tc.tile_pool(name="proj_psum", bufs=2, space="PSUM") as pp:
                # warm the PE clock (HAM) with throwaway matmuls while the
                # input DMAs stream in; results are never read
                warm_ps = pp.tile([128, 512], f32, tag="pj", bufs=8,
                                  name="warm_ps")
                for _ in range(28):
                    nc.tensor.matmul(warm_ps[0:1, :],
                                     vaug_sb[:, 0, HD:HD + 1],
                                     kt_sb[:, 0:512], start=True, stop=True)

                # -- wave 1: K and V (c-outer so PE starts with first chunk) --
                ps_k = [pp.tile([128, 512], f32, tag="pj", bufs=8,
                                name=f"ps_k{j}") for j in range(4)]
                ps_v = [pp.tile([128, 512], f32, tag="pj", bufs=8,
                                name=f"ps_v{j}") for j in range(4)]
                for cch in range(NCH):
                    st, sp = (cch == 0), (cch == NCH - 1)
                    for j in range(T // 512):
                        jsl = slice(512 * j, 512 * (j + 1))
                        nc.tensor.matmul(ps_k[j], wk_sb[:, cch, :],
                                         xt_sb[cch][:, jsl], start=st, stop=sp)
                        nc.tensor.matmul(ps_v[j], wv_sb[:, cch, :],
                                         xt_sb[cch][:, jsl], start=st, stop=sp)
                for j in range(T // 512):
                    jsl = slice(512 * j, 512 * (j + 1))
                    rope_evict(ps_k[j], jsl, kt_sb)
                    nc.scalar.copy(vt_sb[:, jsl], ps_v[j])
                for i in range(NT):
                    nc.sync.dma_start_transpose(
                        vaug_sb[:, i, 0:HD], vt_sb[:, 128 * i:128 * (i + 1)]
                    )

            # ===== phases 1b+2+3: Q proj woven into half-0 attention, =====
            # ===== O-proj rows woven into half-1 attention ================
            # PSUM: tag "s" (2x2 banks) + tag "b1" (3x1, shared by Q-proj
            # accumulators, [Y|Z] groups and O-proj tiles) = 7 banks.
            with tc.tile_pool(name="attn", bufs=1) as ap, \
                 tc.tile_pool(name="attn_psum", bufs=1, space="PSUM") as apsum:

                def q_unit(h, j):
                    hsl = slice(HD * h, HD * (h + 1))
                    jsl = slice(512 * j, 512 * (j + 1))
                    ps_q = apsum.tile([128, 512], f32, tag="b1", bufs=3,
                                      name=f"ps_q{h}_{j}")
                    for cch in range(NCH):
                        nc.tensor.matmul(
                            ps_q, wq_sb[:, cch, hsl], xt_sb[cch][:, jsl],
                            start=(cch == 0), stop=(cch == NCH - 1))
                    rope_evict(ps_q, jsl, qt_sb[h])

                def y_group(h, half, il, tiles, eng=None):
                    """One [Y|Z] accumulation + normalize + transpose-out."""
                    gi = (HW // 128) * half + il
                    ps_yz = apsum.tile([128, 512], f32, tag="b1", bufs=3,
                                       name="ps_yz")
                    for tk in range(gi + 1):
                        nc.tensor.matmul(
                            ps_yz[:, 0:HD + 1],
                            tiles[tk][:, 128 * il:128 * (il + 1)],
                            vaug_sb[:, tk, 0:HD + 1],
                            start=(tk == 0), stop=(tk == gi))
                    rz = ap.tile([128, 1], f32, tag="rz", bufs=8)
                    nc.vector.reciprocal(rz, ps_yz[:, HD:HD + 1])
                    ysb = ap.tile([128, HD], bf, tag="ysb", bufs=8)
                    nc.vector.tensor_scalar_mul(ysb, ps_yz[:, 0:HD], rz)
                    (eng or nc.sync).dma_start_transpose(
                        yt_sb[h][:, 128 * gi:128 * (gi + 1)], ysb)

                def attn_block(h, half, prev, ppool, pbufs, fillers,
                               fill_start, fill_step, junk=False,
                               self_y=False):
                    """S^T + exp for this tq half.  The previous block's
                    Y-groups plus the filler closures (Q-proj units in half
                    0, O-proj tiles in half 1) are woven between the tk
                    iterations so PE always has queued work while ACT
                    catches up on exps."""
                    tq0 = HW * half
                    ntk = (tq0 + HW) // 128
                    tiles = []
                    pi = 0
                    fi = 0

                    def emit_y(n):
                        nonlocal pi
                        while n > 0 and prev is not None and pi < HW // 128:
                            y_group(prev[0], prev[1], pi, prev[2])
                            pi += 1
                            n -= 1

                    for tk in range(ntk):
                        lo = max(0, 128 * tk - tq0)
                        ps_s = apsum.tile([128, HW], f32, tag="s", bufs=2)
                        # filler-less ACT-bound blocks: compute the masked
                        # (junk) columns too so PE/HAM stay warm
                        slo = 0 if junk else lo
                        chunks = ([(slo, 512), (512, HW)] if slo < 512
                                  else [(slo, HW)])
                        for (a, bnd) in chunks:
                            nc.tensor.matmul(
                                ps_s[:, a:bnd],
                                kt_sb[:, 128 * tk:128 * (tk + 1)],
                                qt_sb[h][:, tq0 + a:tq0 + bnd],
                                start=True, stop=True)
                        p_t = ppool.tile([128, HW], bf, tag="p", bufs=pbufs)
                        nc.scalar.activation(p_t[:, lo:HW], ps_s[:, lo:HW],
                                             Exp, scale=SCALE)
                        tiles.append(p_t)
                        if tk >= 1 and (ntk == 8 or tk % 2 == 1):
                            emit_y(1)
                        if (fi < len(fillers) and tk >= fill_start
                                and (tk - fill_start) % fill_step == 0):
                            fillers[fi]()
                            fi += 1
                        if self_y and tk >= 2:
                            # last half-0 block: emit its own Y-groups as
                            # exps complete so the transposes finish
                            # before half 1 needs them
                            y_group(h, half, tk - 2, tiles,
                                    eng=(nc.sync, nc.scalar)[tk % 2])
                    emit_y(HW // 128)
                    while fi < len(fillers):
                        fillers[fi]()
                        fi += 1
                    return tiles

                def oproj_tile(ti, n, dve_only, tailq=False):
                    tsl = slice(128 * ti, 128 * (ti + 1))
                    nsl = slice(512 * n, 512 * (n + 1))
                    ps_o = apsum.tile([128, 512], f32, tag="b1",
                                      bufs=3, name="ps_o")
                    for h in range(NHL):
                        nc.tensor.matmul(
                            ps_o, yt_sb[h][:, tsl], wo_sb[:, h, nsl],
                            start=(h == 0), stop=(h == NHL - 1))
                    ob = ap2.tile([128, 512], bf, tag="ob", bufs=16)
                    if tailq and ti < 12:
                        # early tail: keep DVE free for the wide
                        # reciprocal/normalize chain
                        nc.scalar.copy(ob, ps_o)
                    elif dve_only or (ti * 4 + n) % 2 == 0:
                        nc.vector.tensor_copy(ob, ps_o)
                    else:
                        nc.scalar.copy(ob, ps_o)
                    # tail output drains on two queues in parallel
                    if tailq and n % 2 == 0:
                        nc.sync.dma_start(o[tsl, nsl], ob)
                    else:
                        nc.gpsimd.dma_start(o[tsl, nsl], ob)

                def oproj(t0, t1, dve_only, tailq=False):
                    for ti in range(t0, t1):
                        for n in range(C // 512):
                            oproj_tile(ti, n, dve_only, tailq)

                # ---- Q h0 j0/j1 upfront: a half-0 block only reads its
                # head's first 1024 query columns, so j2/j3 defer into the
                # weave and attention starts two units sooner
                q_unit(0, 0)
                q_unit(0, 1)

                # ---- half 0: each block weaves the next head's j0/j1
                # (needed by the next block) plus its own head's j2/j3
                # (needed from half 1 on) under the ACT-bound exp stretch
                def qf(hh, jj):
                    return lambda: q_unit(hh, jj)

                with tc.tile_pool(name="attn0", bufs=1) as ap0:
                    prev0 = None
                    for h in range(NHL):
                        fil = ([qf(h + 1, 0), qf(h + 1, 1)]
                               if h + 1 < NHL else [])
                        fil += [qf(h, 2), qf(h, 3)]
                        tiles = attn_block(h, 0, prev0, ap0, 16, fil, 1, 2,
                                           self_y=(h == NHL - 1))
                        prev0 = (h, 0, tiles)
                    # groups il 0..5 self-wove into the last block above
                    for il in range(6, HW // 128):
                        y_group(NHL - 1, 0, il, prev0[2],
                                eng=(nc.sync, nc.scalar)[il % 2])
                # x chunks + rope scratch are dead: release them so the
                # half-1 attention pool below can reuse the space
                es1.close()

                ap2_cm = tc.tile_pool(name="attn2", bufs=1)
                ap2 = ap2_cm.__enter__()
                # ---- half 1: head 0 (no prev Y-groups to weave) gets two
                # O-proj rows woven inside + junk columns; heads 1-3 weave
                # the previous head's Y-groups with O-proj rows between
                # blocks, as measured-dense
                fil = [(lambda t, n: lambda: oproj_tile(
                    t, n, dve_only=True))(t, n)
                    for t in (0, 1) for n in range(4)]
                tiles = attn_block(0, 1, None, ap2, 34, fil, 4, 1, junk=True)
                prev = (0, 1, tiles)
                od = 2
                for h in range(1, NHL):
                    tiles = attn_block(h, 1, prev, ap2, 34, [], 0, 1)
                    oproj(od, od + 2, dve_only=True)
                    od += 2
                    prev = (h, 1, tiles)
                # tail: the last head's half-1 Y accumulates V-stationary
                # straight into Y^T (PSUM "s" slots, free once scores end),
                # denominator from an all-ones stationary — no DMA-xbar
                # transposes left between the last exp and the final O-proj.
                tiles3 = prev[2]
                ps_yt = apsum.tile([128, HW], f32, tag="s", bufs=2,
                                   name="ps_yt")
                ps_z = apsum.tile([128, HW], f32, tag="s", bufs=2,
                                  name="ps_z")
                rzw = ap2.tile([128, HW], f32, tag="rzw", bufs=1)

                # fp32 psum writes are one-bank (<=512) wide, so accumulate
                # the two 512-col halves as separate groups; the slow wide
                # reciprocals hide under the other half's matmuls / O-proj
                def yz_chunk(lo0, hi, tkmax):
                    for tk in range(tkmax + 1):
                        a = max(lo0, 128 * tk - HW)
                        st, sp = (tk == 0), (tk == tkmax)
                        nc.tensor.matmul(ps_yt[:, a:hi],
                                         vaug_sb[:, tk, 0:HD],
                                         tiles3[tk][:, a:hi],
                                         start=st, stop=sp)
                        nc.tensor.matmul(ps_z[:, a:hi], ones_sb,
                                         tiles3[tk][:, a:hi],
                                         start=st, stop=sp)

                def norm_piece(piece):
                    psl = slice(512 * piece, 512 * (piece + 1))
                    tsl = slice(HW + 512 * piece, HW + 512 * (piece + 1))
                    nc.vector.tensor_mul(yt_sb[NHL - 1][:, tsl],
                                         ps_yt[:, psl], rzw[:, psl])

                yz_chunk(0, 512, 11)
                nc.vector.reciprocal(rzw[:, 0:512], ps_z[:, 0:512])
                # normalize piece 0 before any chunk-B write is emitted so
                # it can't pick up a same-tile WAR against them
                norm_piece(0)
                yz_chunk(512, HW, NT - 1)
                oproj(od, od + 4, dve_only=False, tailq=True)
                nc.vector.reciprocal(rzw[:, 512:HW], ps_z[:, 512:HW])
                norm_piece(1)
                oproj(od + 4, NT, dve_only=False, tailq=True)
                ap2_cm.__exit__(None, None, None)

    nc.finalize()
    return nc


def _tables():
    freqs = 1.0 / (10000.0 ** (np.arange(0, HD, 2, dtype=np.float32) / HD))
    t = np.arange(T, dtype=np.float32)
    emb = np.outer(t, freqs)                  # [T, 64]
    cos_t = np.cos(emb).T.astype(np.float32)  # [64, T]
    sin_t = np.sin(emb).T.astype(np.float32)
    cosT = np.ascontiguousarray(np.concatenate([cos_t, cos_t], 0)).astype(BF16)
    # halves swapped: row d holds the factor multiplying t0[(d+64)%128]
    # when writing t2[d ^ 64 half]; see rope_evict
    sinT = np.ascontiguousarray(np.concatenate([sin_t, -sin_t], 0)).astype(BF16)
    return cosT, sinT


def _get_nc():
    if "nc" not in _CACHE:
        _CACHE["nc"] = _build_nc()
    return _CACHE["nc"]


def kernel(x, Wq, Wk, Wv, Wo, _trace=False):
    from concourse.bass_utils import run_bass_kernel_spmd

    x = np.asarray(x, dtype=np.float32)
    cosT, sinT = _tables()

    def chunked(w):
        # [K, m] -> [128, K//128, m] (partition-major, contiguous)
        k, m = w.shape
        return np.ascontiguousarray(
            w.reshape(k // 128, 128, m).transpose(1, 0, 2)).astype(BF16)

    in_maps = []
    for core in range(8):
        b, g = divmod(core, 4)
        in_maps.append({
            "xT": np.ascontiguousarray(x[b].T).astype(BF16),
            "wq": chunked(Wq[:, 512 * g:512 * (g + 1)]),
            "wk": chunked(Wk[:, 128 * g:128 * (g + 1)]),
            "wv": chunked(Wv[:, 128 * g:128 * (g + 1)]),
            "wo": chunked(Wo[512 * g:512 * (g + 1), :]),
            "cosT": cosT,
            "sinT": sinT,
        })

    nc = _get_nc()
    res = run_bass_kernel_spmd(nc, in_maps, list(range(8)), trace=_trace)
    parts = [np.asarray(res.results[c]["o_part"], dtype=np.float32)
             for c in range(8)]
    out = np.empty((B, T, C), dtype=np.float32)
    for b in range(B):
        out[b] = parts[4 * b] + parts[4 * b + 1] + parts[4 * b + 2] + parts[4 * b + 3]
    if _trace:
        return out, res
    return out

